# revision 1
# baseline (speedup 1.0000x reference)
"""GAT 3-layer Bass kernel for 8 trn2 cores (dev module)."""
import numpy as np
import concourse.bacc as bacc
import concourse.bass as bass
from concourse import bass_utils
from concourse.tile import TileContext
import concourse.mybir as mybir

N, H, C_OUT, G = 50000, 128, 10, 128
NCORES = 8
NPC = N // NCORES            # 6250
WPC = 49                     # 128-node dst windows per core (last=106)
CHUNK_W = 2
NCHUNK = (WPC + CHUNK_W - 1) // CHUNK_W   # 25
SHARD_PAD = WPC * 128        # 6272
NPAD = SHARD_PAD * NCORES    # 50176
ROW_F16 = 256                # 512B row: [h f16 x128 | as f32 | ad f32 | pad]
LO_ROWS = 32768
EXP_SHIFT = 4.0

F16, F32, I16 = mybir.dt.float16, mybir.dt.float32, mybir.dt.int16
AF = mybir.ActivationFunctionType
OP = mybir.AluOpType


def prep_edges(edge_index):
    """-> chunks_meta, per-core arrays, T_total, n_lo, n_hi.
    chunks_meta[ch] = list of (w, hi, ntile) segments, uniform across cores."""
    src = np.concatenate([edge_index[0], np.arange(N)]).astype(np.int64)
    dst = np.concatenate([edge_index[1], np.arange(N)]).astype(np.int64)
    row_id = (src // NPC) * SHARD_PAD + (src % NPC)

    per_core = []
    for c in range(NCORES):
        m = (dst // NPC) == c
        s_r, d_c = row_id[m], dst[m] - c * NPC
        win = d_c // 128
        core_chunks = []
        for ch in range(NCHUNK):
            wids = [w for w in (2 * ch, 2 * ch + 1) if w < WPC]
            segs = {}
            for hi in (0, 1):
                for w in wids:
                    mm = (win == w) & ((s_r >= LO_ROWS) == bool(hi))
                    rows = s_r[mm]
                    o = np.argsort(rows, kind="stable")
                    segs[(w, hi)] = (rows[o], (d_c[mm] - w * 128)[o])
            core_chunks.append(segs)
        per_core.append(core_chunks)

    chunks_meta = []
    for ch in range(NCHUNK):
        meta = []
        for key in per_core[0][ch]:
            w, hi = key
            mx = max(len(per_core[c][ch][key][0]) for c in range(NCORES))
            meta.append((w, hi, max(1, -(-mx // 128))))
        meta.sort(key=lambda x: (x[1], x[0]))  # lo segs first, then hi
        chunks_meta.append(meta)

    idx_lo = [[] for _ in range(NCORES)]
    idx_hi = [[] for _ in range(NCORES)]
    dstl = [[] for _ in range(NCORES)]
    for ch in range(NCHUNK):
        for (w, hi, ntile) in chunks_meta[ch]:
            L = ntile * 128
            for c in range(NCORES):
                rows, dl = per_core[c][ch][(w, hi)]
                r = np.zeros(L, np.int64)
                d = np.full(L, -1.0, np.float32)
                r[: len(rows)] = rows - (LO_ROWS if hi else 0)
                d[: len(dl)] = dl
                (idx_hi if hi else idx_lo)[c].append(r)
                dstl[c].append(d)

    def wrap16(a):
        a = a.astype(np.int16).reshape(-1, 16).T
        return np.tile(a, (8, 1)).copy()

    cores = []
    for c in range(NCORES):
        lo = np.concatenate(idx_lo[c]); hi = np.concatenate(idx_hi[c])
        dl = np.concatenate(dstl[c])
        dstl_pt = dl.reshape(-1, 128).T.astype(np.float16).copy()  # [128(edge), T]
        T = dstl_pt.shape[1]
        # m0t[n, t*128+j] = 1 if dstl[j, t] == n  (fp8 one-hot, transposed)
        dmat = dl.reshape(T, 128)  # [t, j]
        m0t = (np.arange(128)[:, None, None] == dmat[None, :, :]).reshape(128, T * 128)
        cores.append(dict(
            idxlo=wrap16(lo), idxhi=wrap16(hi),
            dstl=dstl_pt, m0t=m0t.astype(mybir.dt.np(mybir.dt.float8e4)),
        ))
    T_total = sum(nt for ch in chunks_meta for (_, _, nt) in ch)
    n_lo = sum(nt * 128 for ch in chunks_meta for (_, hi, nt) in ch if not hi)
    n_hi = sum(nt * 128 for ch in chunks_meta for (_, hi, nt) in ch if hi)
    return chunks_meta, cores, T_total, n_lo, n_hi


def make_weight_inputs(W1, a_src1, a_dst1, b1, W2, a_src2, a_dst2, b2,
                       W3, a_src3, a_dst3, b3, lin_W, lin_b, x):
    """Host-side constant tensors (replicated to all cores)."""
    waug = np.zeros((128, 3, 130), np.float16)
    brep = np.zeros((128, 3, 128), np.float16)
    for i, (W, asr, ads, b) in enumerate([(W1, a_src1, a_dst1, b1),
                                          (W2, a_src2, a_dst2, b2),
                                          (W3, a_src3, a_dst3, b3)]):
        waug[:, i, 0:128] = W.astype(np.float32)
        waug[:, i, 128] = (W.astype(np.float64) @ asr.astype(np.float64)).astype(np.float32)
        waug[:, i, 129] = (W.astype(np.float64) @ ads.astype(np.float64)).astype(np.float32)
        brep[:, i, :] = np.broadcast_to(b.astype(np.float32), (128, 128))
    xT = np.zeros((128, NPAD), np.float16)
    xv = x.astype(np.float16)
    for r in range(NCORES):
        xT[:, r * SHARD_PAD: r * SHARD_PAD + NPC] = xv[r * NPC:(r + 1) * NPC].T
    iota = np.broadcast_to(np.arange(128, dtype=np.float16), (128, 128)).copy()
    return dict(
        xT=xT, waug=waug, brep=brep,
        linw=lin_W.astype(np.float16),
        linb=np.broadcast_to(lin_b.astype(np.float32), (128, C_OUT)).copy(),
        iota=iota, idm=np.eye(128, dtype=np.float16),
    )


def make_xsT0(x, core):
    out = np.zeros((128, SHARD_PAD), np.float16)
    out[:, :NPC] = x[core * NPC:(core + 1) * NPC].astype(np.float16).T
    return out


def make_batch_input(batch, core):
    bl = np.full((128, WPC), -1.0, np.float32)
    ids = batch[core * NPC:(core + 1) * NPC].astype(np.float32)
    for w in range(WPC):
        seg = ids[w * 128:(w + 1) * 128]
        bl[: len(seg), w] = seg
    return bl


def split_waits(nc, maxw=1):
    n = 0
    for func in nc.m.functions:
        for block in func.blocks:
            new = []
            for inst in block.instructions:
                si = inst.sync_info
                if si is not None and si.on_wait and len(si.on_wait) > maxw:
                    w = list(si.on_wait); extra, keep = w[:-maxw], w[-maxw:]
                    while extra:
                        ck, extra = extra[:maxw], extra[maxw:]
                        new.append(mybir.InstNoOp(name=f"ws-{n}", engine=inst.engine,
                            sync_info=mybir.SyncInfo(on_wait=ck, on_update=[])))
                        n += 1
                    si.on_wait = keep
                new.append(inst)
            block.instructions = new
    return n


def build(nc, chunks_meta, T_total, n_lo, n_hi, n_layers=3, with_pool=True, dump_xsT=False, do_edge=True, dump_htab=0, edge_mode=4, dump_g=False, dump_dbg=False):
    xT_in   = nc.dram_tensor("xT", [128, NPAD], F16, kind="ExternalInput")
    waug_in = nc.dram_tensor("waug", [128, 3, 130], F16, kind="ExternalInput")
    brep_in = nc.dram_tensor("brep", [128, 3, 128], F16, kind="ExternalInput")
    linw_in = nc.dram_tensor("linw", [128, C_OUT], F16, kind="ExternalInput")
    linb_in = nc.dram_tensor("linb", [128, C_OUT], F32, kind="ExternalInput")
    iota_in = nc.dram_tensor("iota", [128, 128], F16, kind="ExternalInput")
    idm_in  = nc.dram_tensor("idm", [128, 128], F16, kind="ExternalInput")
    bl_in   = nc.dram_tensor("batchl", [128, WPC], F32, kind="ExternalInput")
    ilo_in  = nc.dram_tensor("idxlo", [128, n_lo // 16], I16, kind="ExternalInput")
    ihi_in  = nc.dram_tensor("idxhi", [128, n_hi // 16], I16, kind="ExternalInput")
    dstl_in = nc.dram_tensor("dstl", [128, T_total], F16, kind="ExternalInput")
    m0t_in  = nc.dram_tensor("m0t", [128, T_total * 128], mybir.dt.float8e4, kind="ExternalInput")
    xsT0_in = nc.dram_tensor("xsT0", [128, SHARD_PAD], F16, kind="ExternalInput")
    out_t   = nc.dram_tensor("out", [G, C_OUT], F32, kind="ExternalOutput")
    xsT_out = nc.dram_tensor("xsT_out", [128, SHARD_PAD], F16, kind="ExternalOutput") if dump_xsT else None
    htab_out = nc.dram_tensor("htab_out", [dump_htab, ROW_F16], F16, kind="ExternalOutput") if dump_htab else None
    CT0 = sum(nt for (_, _, nt) in chunks_meta[0])
    g_out = nc.dram_tensor("g_out", [128, CT0, ROW_F16], F16, kind="ExternalOutput") if dump_g else None
    ef_out = nc.dram_tensor("ef_out", [128, CT0], F16, kind="ExternalOutput") if dump_dbg else None
    m0_out = nc.dram_tensor("m0_out", [128, CT0, 128], F16, kind="ExternalOutput") if dump_dbg else None
    ps_out = nc.dram_tensor("ps_out", [128, 129], F32, kind="ExternalOutput") if dump_dbg else None
    adl_out = nc.dram_tensor("adl_out", [128, WPC], F16, kind="ExternalOutput") if dump_dbg else None
    adx_out = nc.dram_tensor("adx_out", [128, 512], F32, kind="ExternalOutput") if dump_dbg else None

    CT_MAX = max(sum(nt for (_, _, nt) in ch) for ch in chunks_meta)

    with TileContext(nc) as tc:
        with tc.tile_pool(name="const", bufs=1) as constp, \
             tc.tile_pool(name="xTp", bufs=1) as xtp, \
             tc.tile_pool(name="gath", bufs=2) as gathp, \
             tc.tile_pool(name="m0p", bufs=2) as m0p, \
             tc.tile_pool(name="ewp", bufs=2) as ewp, \
             tc.tile_pool(name="evac", bufs=3) as evp, \
             tc.tile_pool(name="stage", bufs=2) as stp, \
             tc.tile_pool(name="psw", bufs=2, space="PSUM") as psw, \
             tc.tile_pool(name="pst", bufs=2, space="PSUM") as pst, \
             tc.tile_pool(name="pstr", bufs=1, space="PSUM") as pstr, \
             tc.tile_pool(name="psp", bufs=1, space="PSUM") as psp, \
             tc.tile_pool(name="psadx", bufs=1, space="PSUM") as psadx, \
             tc.tile_pool(name="psadl", bufs=1, space="PSUM") as psadl, \
             tc.tile_pool(name="m0tp", bufs=1) as m0tp, \
             tc.tile_pool(name="dram", bufs=1, space="DRAM") as dram:

            xT   = xtp.tile([128, NPAD], F16)
            xsT  = xtp.tile([128, SHARD_PAD], F16)   # own-shard transposed output
            waug = constp.tile([128, 3, 130], F16)
            brep = constp.tile([128, 3, 128], F16)
            linw = constp.tile([128, C_OUT], F16)
            linb = constp.tile([128, C_OUT], F32)
            iota = constp.tile([128, 128], F16)
            idm  = constp.tile([128, 128], F16)
            bl   = constp.tile([128, WPC], F32)
            ilo  = constp.tile([128, n_lo // 16], I16)
            ihi  = constp.tile([128, n_hi // 16], I16)
            dstl = constp.tile([128, T_total], F16)
            nc.sync.dma_start(out=xsT[:], in_=xsT0_in[:])
            for t, s in [(xT, xT_in), (waug, waug_in), (brep, brep_in),
                         (linw, linw_in), (linb, linb_in), (iota, iota_in),
                         (idm, idm_in), (bl, bl_in), (ilo, ilo_in),
                         (ihi, ihi_in), (dstl, dstl_in)]:
                nc.sync.dma_start(out=t[:], in_=s[:])

            negshift = constp.tile([128, 1], F32)
            nc.vector.memset(negshift[:], -EXP_SHIFT)
            htab = dram.tile([NPAD, ROW_F16], F16)
            pool_bi = dram.tile([128, 129], F32)
            pool_bo = dram.tile([128, 129], F32, addr_space="Shared")

            # =========================================================
            def table_build(layer):
                BATCH = 4
                nchunks = NPAD // 128
                for b0 in range(0, nchunks, BATCH):
                    bn = min(BATCH, nchunks - b0)
                    stg = stp.tile([128, BATCH, ROW_F16], F16, tag="stg")
                    stg32 = stg[:].bitcast(F32)
                    nc.vector.memset(stg[:, :, 132:256], 0.0)
                    for j in range(bn):
                        cid = b0 + j
                        ps = pst.tile([128, 130], F32, tag="tab")
                        nc.tensor.matmul(ps[:], xT[:, cid * 128:(cid + 1) * 128],
                                         waug[:, layer, :], start=True, stop=True,
                                         skip_group_check=True)
                        if j % 2 == 0:
                            nc.vector.tensor_copy(stg[:, j, 0:128], ps[:, 0:128])
                        else:
                            nc.scalar.activation(stg[:, j, 0:128], ps[:, 0:128], AF.Copy)
                        nc.vector.tensor_copy(stg32[:, j, 64:66], ps[:, 128:130])
                    nc.sync.dma_start(
                        out=htab[b0 * 128:(b0 + bn) * 128, :]
                            .rearrange("(b p) e -> p b e", p=128),
                        in_=stg[:, 0:bn, :])

            # =========================================================
            def edge_phase(layer):
                # per-window a_dst . x_own via tiny matmuls over xsT
                adlps = psadl.tile([128, WPC], F32, tag="adl", name=f"adlps_{layer}")
                for w in range(WPC):
                    nc.tensor.matmul(adlps[:, w:w + 1], xsT[:, w * 128:(w + 1) * 128],
                                     waug[:, layer, 129:130], start=True, stop=True,
                                     skip_group_check=True)
                adl16 = ewp.tile([128, WPC], F16, tag="adl16", name=f"adl16_{layer}", bufs=1)
                nc.scalar.activation(adl16[:], adlps[:], AF.Copy)
                if dump_dbg and layer == 0:
                    nc.sync.dma_start(out=adl_out[:], in_=adl16[:])
                t0 = 0; off_lo = 0; off_hi = 0
                pool_ps = psp.tile([128, 129], F32, tag="pool", name="pool_ps") if (with_pool and layer == n_layers - 1) else None
                for ch, meta in enumerate(chunks_meta):
                    ct = sum(nt for (_, _, nt) in meta)
                    gt = gathp.tile([128, CT_MAX, ROW_F16], F16, tag="g")
                    tt = 0
                    for want_hi in (0, 1):
                        n_seg = sum(nt for (_, hi, nt) in meta if hi == want_hi) * 128
                        if n_seg == 0:
                            continue
                        src_ap = htab[LO_ROWS:NPAD, :] if want_hi else htab[0:LO_ROWS, :]
                        if want_hi:
                            idxs = ihi[:, off_hi // 16:(off_hi + n_seg) // 16]
                            off_hi += n_seg
                        else:
                            idxs = ilo[:, off_lo // 16:(off_lo + n_seg) // 16]
                            off_lo += n_seg
                        nc.gpsimd.dma_gather(
                            out_ap=gt[:, tt:tt + n_seg // 128, :], in_ap=src_ap,
                            idxs_ap=idxs, num_idxs=n_seg, num_idxs_reg=n_seg,
                            elem_size=ROW_F16, single_packet=False)
                        tt += n_seg // 128
                    if ch == 0 and dump_g is not False and g_out is not None:
                        nc.sync.dma_start(out=g_out[:], in_=gt[:, 0:ct, :])
                    if edge_mode < 2:
                        t0 += ct
                        continue
                    m0t = m0tp.tile([128, CT_MAX * 128], mybir.dt.float8e4, tag="m0t")
                    nc.sync.dma_start(out=m0t[:, 0:ct * 128],
                                      in_=m0t_in[:, t0 * 128:(t0 + ct) * 128])
                    adx = psadx.tile([128, 512], F32, tag="adx", name=f"adx_{layer}_{ch}")
                    # first/last tile per window (also used for expand rhs)
                    ftw = {}
                    _tt = 0
                    for (w, hi, nt) in meta:
                        for _ in range(nt):
                            ftw[_tt] = w
                            _tt += 1
                    for _tt in range(ct):
                        nc.tensor.matmul(adx[:, _tt:_tt + 1],
                                         m0t[:, _tt * 128:(_tt + 1) * 128],
                                         adl16[:, ftw[_tt]:ftw[_tt] + 1],
                                         start=True, stop=True, skip_group_check=True)
                    g32 = gt[:].bitcast(F32)
                    z  = ewp.tile([128, CT_MAX], F32, tag="z")
                    e1 = ewp.tile([128, CT_MAX], F32, tag="e1")
                    ef = ewp.tile([128, CT_MAX], F16, tag="ef")
                    nc.vector.tensor_tensor(z[:, 0:ct].unsqueeze(2),
                                            g32[:, 0:ct, 64:65], adx[:, 0:ct].unsqueeze(2), OP.add)
                    nc.scalar.activation(e1[:, 0:ct], z[:, 0:ct], AF.Exp, bias=negshift[:])
                    nc.scalar.activation(z[:, 0:ct], z[:, 0:ct], AF.Exp, bias=negshift[:], scale=0.2)
                    nc.vector.tensor_tensor(ef[:, 0:ct], e1[:, 0:ct], z[:, 0:ct], OP.max)
                    nc.vector.memset(gt[:, 0:ct, 128:130], 1.0)
                    nc.vector.tensor_tensor(
                        gt[:, 0:ct, 0:130], gt[:, 0:ct, 0:130],
                        ef[:, 0:ct].unsqueeze(2).to_broadcast((128, ct, 130)), OP.mult)
                    if edge_mode < 3:
                        t0 += ct
                        continue
                    m0 = m0p.tile([128, CT_MAX, 128], F16, tag="m0")
                    nc.vector.tensor_tensor(
                        m0[:, 0:ct, :],
                        iota[:].unsqueeze(1).to_broadcast((128, ct, 128)),
                        dstl[:, t0:t0 + ct].unsqueeze(2).to_broadcast((128, ct, 128)),
                        OP.is_equal)
                    if dump_dbg and ch == 0:
                        acp = stp.tile([128, 512], F32, tag="stg", name="acp")
                        nc.vector.memset(acp[:], 0.0)
                        nc.vector.tensor_copy(acp[:, 0:ct], adx[:, 0:ct])
                        nc.sync.dma_start(out=adx_out[:], in_=acp[:])
                        nc.sync.dma_start(out=ef_out[:], in_=ef[:, 0:ct])
                        nc.sync.dma_start(out=m0_out[:], in_=m0[:, 0:ct, :])
                    # first/last tile per window
                    ft, lt = {}, {}
                    tt = 0
                    for (w, hi, nt) in meta:
                        for _ in range(nt):
                            if w not in ft: ft[w] = tt
                            lt[w] = tt
                            tt += 1
                    psws = {w: psw.tile([128, 129], F32, tag="win", name=f"win_{layer}_{ch}_{w}") for w in ft}
                    tt = 0
                    for (w, hi, nt) in meta:
                        for _ in range(nt):
                            nc.tensor.matmul(psws[w][:], m0[:, tt, :], gt[:, tt, 0:129],
                                             start=(tt == ft[w]), stop=(tt == lt[w]),
                                             skip_group_check=True)
                            tt += 1
                    if edge_mode < 4:
                        t0 += ct
                        continue
                    for w in sorted(ft):
                        ps = psws[w]
                        if dump_dbg and ch == 0 and w == 0:
                            pcp = evp.tile([128, 129], F32, tag="pcp", name="pcp")
                            nc.vector.tensor_copy(pcp[:], ps[:])
                            nc.sync.dma_start(out=ps_out[:], in_=pcp[:])
                        dn = evp.tile([128, 1], F32, tag="dn")
                        nc.vector.tensor_scalar_max(dn[:], ps[:, 128:129], 1e-6)
                        rc = evp.tile([128, 1], F32, tag="rc")
                        nc.vector.reciprocal(rc[:], dn[:])
                        xw = evp.tile([128, 128], F16, tag="xw")
                        nc.scalar.activation(xw[:], ps[:, 0:128], AF.Copy, scale=rc[:])
                        nc.vector.tensor_tensor(xw[:], xw[:], brep[:, layer, :], OP.add)
                        nc.vector.tensor_scalar_max(xw[:], xw[:], 0.0)
                        if pool_ps is None:
                            tp = pstr.tile([128, 128], F16, tag="tr")
                            nc.tensor.transpose(tp[:], xw[:], idm[:])
                            nc.vector.tensor_copy(xsT[:, w * 128:(w + 1) * 128], tp[:])
                        else:
                            ob = evp.tile([128, 128], F16, tag="ob")
                            nc.vector.tensor_scalar(ob[:], iota[:], bl[:, w:w + 1], None,
                                                    OP.is_equal)
                            x1 = evp.tile([128, 129], F16, tag="x1")
                            nc.vector.tensor_copy(x1[:, 0:128], xw[:])
                            nc.vector.memset(x1[:, 128:129], 1.0)
                            nc.tensor.matmul(pool_ps[:], ob[:], x1[:],
                                             start=(w == 0), stop=(w == WPC - 1),
                                             skip_group_check=True)
                    t0 += ct
                return pool_ps

            # ================= main =================
            for layer in range(n_layers):
                if layer > 0:
                    bounce_in = dram.tile([128, SHARD_PAD], F16, name=f"bi_{layer}", tag=f"bi_{layer}")
                    bounce_out = dram.tile([NCORES, 128, SHARD_PAD], F16, addr_space="Shared", name=f"bo_{layer}", tag=f"bo_{layer}")
                    nc.sync.dma_start(out=bounce_in[:], in_=xsT[:])
                    nc.gpsimd.collective_compute(
                        "AllGather", OP.bypass, replica_groups=[list(range(NCORES))],
                        ins=[bounce_in[:].opt()], outs=[bounce_out[:].opt()])
                    nc.sync.dma_start(out=xT[:].rearrange("p (r c) -> p r c", r=NCORES),
                                      in_=bounce_out[:].rearrange("r p c -> p r c"))
                table_build(layer)
                pool_ps = edge_phase(layer) if do_edge else None

            if dump_htab:
                hcp = gathp.tile([128, dump_htab // 128, ROW_F16], F16, tag="g", name="hcp")
                nc.sync.dma_start(out=hcp[:], in_=htab[0:dump_htab, :].rearrange("(b p) e -> p b e", p=128))
                nc.sync.dma_start(out=htab_out[:].rearrange("(b p) e -> p b e", p=128), in_=hcp[:])
            if dump_xsT:
                nc.sync.dma_start(out=xsT_out[:], in_=xsT[:])
            if not with_pool:
                zz = evp.tile([128, C_OUT], F32, tag="res")
                nc.vector.memset(zz[:], 0.0)
                nc.sync.dma_start(out=out_t[:], in_=zz[:])
                return nc
            pooled = evp.tile([128, 129], F32, tag="pooled")
            nc.vector.tensor_copy(pooled[:], pool_ps[:])
            nc.sync.dma_start(out=pool_bi[:], in_=pooled[:])
            nc.gpsimd.collective_compute(
                "AllReduce", OP.add, replica_groups=[list(range(NCORES))],
                ins=[pool_bi[:].opt()], outs=[pool_bo[:].opt()])
            nc.sync.dma_start(out=pooled[:], in_=pool_bo[:])
            cnt = evp.tile([128, 1], F32, tag="cnt")
            nc.vector.tensor_scalar_max(cnt[:], pooled[:, 128:129], 1.0)
            rcn = evp.tile([128, 1], F32, tag="rcn")
            nc.vector.reciprocal(rcn[:], cnt[:])
            pm = evp.tile([128, 128], F16, tag="pm")
            nc.scalar.activation(pm[:], pooled[:, 0:128], AF.Copy, scale=rcn[:])
            pt = pstr.tile([128, 128], F16, tag="tr")
            nc.tensor.transpose(pt[:], pm[:], idm[:])
            pts = evp.tile([128, 128], F16, tag="pts")
            nc.vector.tensor_copy(pts[:], pt[:])
            ho = psw.tile([128, 129], F32, tag="win")
            nc.tensor.matmul(ho[:, 0:C_OUT], pts[:], linw[:], start=True, stop=True,
                             skip_group_check=True)
            res = evp.tile([128, C_OUT], F32, tag="res")
            nc.vector.tensor_tensor(res[:], ho[:, 0:C_OUT], linb[:], OP.add)
            nc.sync.dma_start(out=out_t[:], in_=res[:])
    return nc


def run(inputs, trace=False, n_layers=3, with_pool=True, dump_xsT=False, do_edge=True, dump_htab=0, edge_mode=4, dump_g=False, dump_dbg=False):
    """Full pipeline: host prep -> build -> run on 8 cores -> [G, C_OUT] f32."""
    chunks_meta, cores, T_total, n_lo, n_hi = prep_edges(np.asarray(inputs["edge_index"]))
    const_ins = make_weight_inputs(
        np.asarray(inputs["W1"]), np.asarray(inputs["a_src1"]), np.asarray(inputs["a_dst1"]), np.asarray(inputs["b1"]),
        np.asarray(inputs["W2"]), np.asarray(inputs["a_src2"]), np.asarray(inputs["a_dst2"]), np.asarray(inputs["b2"]),
        np.asarray(inputs["W3"]), np.asarray(inputs["a_src3"]), np.asarray(inputs["a_dst3"]), np.asarray(inputs["b3"]),
        np.asarray(inputs["lin_W"]), np.asarray(inputs["lin_b"]), np.asarray(inputs["x"]))
    batch = np.asarray(inputs["batch"])

    nc = bacc.Bacc("TRN2", target_bir_lowering=False, debug=False, num_devices=NCORES)
    build(nc, chunks_meta, T_total, n_lo, n_hi, n_layers=n_layers, with_pool=with_pool, dump_xsT=dump_xsT, do_edge=do_edge, dump_htab=dump_htab, edge_mode=edge_mode, dump_g=dump_g, dump_dbg=dump_dbg)
    nc.compile()
    split_waits(nc)

    in_maps = []
    for c in range(NCORES):
        m = dict(const_ins)
        m["batchl"] = make_batch_input(batch, c)
        m["idxlo"] = cores[c]["idxlo"]
        m["m0t"] = cores[c]["m0t"]
        m["xsT0"] = make_xsT0(np.asarray(inputs["x"]), c)
        m["idxhi"] = cores[c]["idxhi"]
        m["dstl"] = cores[c]["dstl"]
        in_maps.append(m)
    res = bass_utils.run_bass_kernel_spmd(nc, in_maps, core_ids=list(range(NCORES)),
                                          trace=trace)
    return res.results[0], res


def kernel(**inputs):
    """Harness entry: full unsharded inputs -> [128, 10] fp32 output."""
    out, _ = run(inputs)
    if isinstance(out, dict):
        out = out["out"]
    return np.asarray(out, dtype=np.float32)



# revision 4
# speedup vs baseline: 1.1706x; 1.1706x over previous
"""GAT 3-layer Bass kernel for 8 trn2 cores (dev module)."""
import numpy as np
import concourse.bacc as bacc
import concourse.bass as bass
from concourse import bass_utils
from concourse.tile import TileContext
import concourse.mybir as mybir

N, H, C_OUT, G = 50000, 128, 10, 128
NCORES = 8
NPC = N // NCORES            # 6250
WPC = 49                     # 128-node dst windows per core (last=106)
CHUNK_W = 2
NCHUNK = (WPC + CHUNK_W - 1) // CHUNK_W   # 25
NQUEUES = 4                  # SWDGE queues: gather desc-gen across Q7 pairs
SHARD_PAD = WPC * 128        # 6272
NPAD = SHARD_PAD * NCORES    # 50176
ROW_F16 = 256                # 512B row: [h f16 x128 | as f32 | ad f32 | pad]
LO_ROWS = 32768
EXP_SHIFT = 4.0

F16, F32, I16 = mybir.dt.float16, mybir.dt.float32, mybir.dt.int16
AF = mybir.ActivationFunctionType
OP = mybir.AluOpType


def prep_edges(edge_index):
    """-> chunks_meta, per-core arrays, T_total, n_lo, n_hi.
    chunks_meta[ch] = list of (w, hi, ntile) segments, uniform across cores."""
    src = np.concatenate([edge_index[0], np.arange(N)]).astype(np.int64)
    dst = np.concatenate([edge_index[1], np.arange(N)]).astype(np.int64)
    row_id = (src // NPC) * SHARD_PAD + (src % NPC)

    per_core = []
    for c in range(NCORES):
        m = (dst // NPC) == c
        s_r, d_c = row_id[m], dst[m] - c * NPC
        win = d_c // 128
        core_chunks = []
        for ch in range(NCHUNK):
            wids = [w for w in (2 * ch, 2 * ch + 1) if w < WPC]
            segs = {}
            for hi in (0, 1):
                for w in wids:
                    mm = (win == w) & ((s_r >= LO_ROWS) == bool(hi))
                    rows = s_r[mm]
                    o = np.argsort(rows, kind="stable")
                    segs[(w, hi)] = (rows[o], (d_c[mm] - w * 128)[o])
            core_chunks.append(segs)
        per_core.append(core_chunks)

    chunks_meta = []
    for ch in range(NCHUNK):
        meta = []
        for key in per_core[0][ch]:
            w, hi = key
            mx = max(len(per_core[c][ch][key][0]) for c in range(NCORES))
            meta.append((w, hi, max(1, -(-mx // 128))))
        meta.sort(key=lambda x: (x[1], x[0]))  # lo segs first, then hi
        chunks_meta.append(meta)

    idx_lo = [[] for _ in range(NCORES)]
    idx_hi = [[] for _ in range(NCORES)]
    dstl = [[] for _ in range(NCORES)]
    for ch in range(NCHUNK):
        for (w, hi, ntile) in chunks_meta[ch]:
            L = ntile * 128
            for c in range(NCORES):
                rows, dl = per_core[c][ch][(w, hi)]
                r = np.zeros(L, np.int64)
                d = np.full(L, -1.0, np.float32)
                r[: len(rows)] = rows - (LO_ROWS if hi else 0)
                d[: len(dl)] = dl
                (idx_hi if hi else idx_lo)[c].append(r)
                dstl[c].append(d)

    def wrap16(a):
        a = a.astype(np.int16).reshape(-1, 16).T
        return np.tile(a, (8, 1)).copy()

    cores = []
    for c in range(NCORES):
        lo = np.concatenate(idx_lo[c]); hi = np.concatenate(idx_hi[c])
        dl = np.concatenate(dstl[c])
        dstl_pt = dl.reshape(-1, 128).T.astype(np.float16).copy()  # [128(edge), T]
        T = dstl_pt.shape[1]
        # m0t[n, t*128+j] = 1 if dstl[j, t] == n  (fp8 one-hot, transposed)
        dmat = dl.reshape(T, 128)  # [t, j]
        m0t = (np.arange(128)[:, None, None] == dmat[None, :, :]).reshape(128, T * 128)
        cores.append(dict(
            idxlo=wrap16(lo), idxhi=wrap16(hi),
            dstl=dstl_pt, m0t=m0t.astype(mybir.dt.np(mybir.dt.float8e4)),
        ))
    T_total = sum(nt for ch in chunks_meta for (_, _, nt) in ch)
    n_lo = sum(nt * 128 for ch in chunks_meta for (_, hi, nt) in ch if not hi)
    n_hi = sum(nt * 128 for ch in chunks_meta for (_, hi, nt) in ch if hi)
    return chunks_meta, cores, T_total, n_lo, n_hi


def make_weight_inputs(W1, a_src1, a_dst1, b1, W2, a_src2, a_dst2, b2,
                       W3, a_src3, a_dst3, b3, lin_W, lin_b, x):
    """Host-side constant tensors (replicated to all cores)."""
    waug = np.zeros((128, 3, 130), np.float16)
    brep = np.zeros((128, 3, 128), np.float16)
    for i, (W, asr, ads, b) in enumerate([(W1, a_src1, a_dst1, b1),
                                          (W2, a_src2, a_dst2, b2),
                                          (W3, a_src3, a_dst3, b3)]):
        waug[:, i, 0:128] = W.astype(np.float32)
        waug[:, i, 128] = (W.astype(np.float64) @ asr.astype(np.float64)).astype(np.float32)
        waug[:, i, 129] = (W.astype(np.float64) @ ads.astype(np.float64)).astype(np.float32)
        brep[:, i, :] = np.broadcast_to(b.astype(np.float32), (128, 128))
    xT = np.zeros((128, NPAD), np.float16)
    xv = x.astype(np.float16)
    for r in range(NCORES):
        xT[:, r * SHARD_PAD: r * SHARD_PAD + NPC] = xv[r * NPC:(r + 1) * NPC].T
    iota = np.broadcast_to(np.arange(128, dtype=np.float16), (128, 128)).copy()
    return dict(
        xT=xT, waug=waug, brep=brep,
        linw=lin_W.astype(np.float16),
        linb=np.broadcast_to(lin_b.astype(np.float32), (128, C_OUT)).copy(),
        iota=iota, idm=np.eye(128, dtype=np.float16),
    )


def make_xsT0(x, core):
    out = np.zeros((128, SHARD_PAD), np.float16)
    out[:, :NPC] = x[core * NPC:(core + 1) * NPC].astype(np.float16).T
    return out


def make_batch_input(batch, core):
    bl = np.full((128, WPC), -1.0, np.float32)
    ids = batch[core * NPC:(core + 1) * NPC].astype(np.float32)
    for w in range(WPC):
        seg = ids[w * 128:(w + 1) * 128]
        bl[: len(seg), w] = seg
    return bl


def split_waits(nc, maxw=1):
    n = 0
    for func in nc.m.functions:
        for block in func.blocks:
            new = []
            for inst in block.instructions:
                si = inst.sync_info
                if si is not None and si.on_wait and len(si.on_wait) > maxw:
                    w = list(si.on_wait); extra, keep = w[:-maxw], w[-maxw:]
                    while extra:
                        ck, extra = extra[:maxw], extra[maxw:]
                        new.append(mybir.InstNoOp(name=f"ws-{n}", engine=inst.engine,
                            sync_info=mybir.SyncInfo(on_wait=ck, on_update=[])))
                        n += 1
                    si.on_wait = keep
                new.append(inst)
            block.instructions = new
    return n


def build(nc, chunks_meta, T_total, n_lo, n_hi, n_layers=3, with_pool=True, dump_xsT=False, do_edge=True, dump_htab=0, edge_mode=4, dump_g=False, dump_dbg=False):
    xT_in   = nc.dram_tensor("xT", [128, NPAD], F16, kind="ExternalInput")
    waug_in = nc.dram_tensor("waug", [128, 3, 130], F16, kind="ExternalInput")
    brep_in = nc.dram_tensor("brep", [128, 3, 128], F16, kind="ExternalInput")
    linw_in = nc.dram_tensor("linw", [128, C_OUT], F16, kind="ExternalInput")
    linb_in = nc.dram_tensor("linb", [128, C_OUT], F32, kind="ExternalInput")
    iota_in = nc.dram_tensor("iota", [128, 128], F16, kind="ExternalInput")
    idm_in  = nc.dram_tensor("idm", [128, 128], F16, kind="ExternalInput")
    bl_in   = nc.dram_tensor("batchl", [128, WPC], F32, kind="ExternalInput")
    ilo_in  = nc.dram_tensor("idxlo", [128, n_lo // 16], I16, kind="ExternalInput")
    ihi_in  = nc.dram_tensor("idxhi", [128, n_hi // 16], I16, kind="ExternalInput")
    dstl_in = nc.dram_tensor("dstl", [128, T_total], F16, kind="ExternalInput")
    m0t_in  = nc.dram_tensor("m0t", [128, T_total * 128], mybir.dt.float8e4, kind="ExternalInput")
    xsT0_in = nc.dram_tensor("xsT0", [128, SHARD_PAD], F16, kind="ExternalInput")
    out_t   = nc.dram_tensor("out", [G, C_OUT], F32, kind="ExternalOutput")
    xsT_out = nc.dram_tensor("xsT_out", [128, SHARD_PAD], F16, kind="ExternalOutput") if dump_xsT else None
    htab_out = nc.dram_tensor("htab_out", [dump_htab, ROW_F16], F16, kind="ExternalOutput") if dump_htab else None
    CT0 = sum(nt for (_, _, nt) in chunks_meta[0])
    g_out = nc.dram_tensor("g_out", [128, CT0, ROW_F16], F16, kind="ExternalOutput") if dump_g else None
    ef_out = nc.dram_tensor("ef_out", [128, CT0], F16, kind="ExternalOutput") if dump_dbg else None
    m0_out = nc.dram_tensor("m0_out", [128, CT0, 128], F16, kind="ExternalOutput") if dump_dbg else None
    ps_out = nc.dram_tensor("ps_out", [128, 129], F32, kind="ExternalOutput") if dump_dbg else None
    adl_out = nc.dram_tensor("adl_out", [128, WPC], F16, kind="ExternalOutput") if dump_dbg else None
    adx_out = nc.dram_tensor("adx_out", [128, 512], F32, kind="ExternalOutput") if dump_dbg else None

    CT_MAX = max(sum(nt for (_, _, nt) in ch) for ch in chunks_meta)

    with TileContext(nc) as tc:
        with tc.tile_pool(name="const", bufs=1) as constp, \
             tc.tile_pool(name="xTp", bufs=1) as xtp, \
             tc.tile_pool(name="gath", bufs=2) as gathp, \
             tc.tile_pool(name="m0p", bufs=2) as m0p, \
             tc.tile_pool(name="ewp", bufs=2) as ewp, \
             tc.tile_pool(name="evac", bufs=3) as evp, \
             tc.tile_pool(name="stage", bufs=2) as stp, \
             tc.tile_pool(name="psw", bufs=2, space="PSUM") as psw, \
             tc.tile_pool(name="pst", bufs=2, space="PSUM") as pst, \
             tc.tile_pool(name="pstr", bufs=1, space="PSUM") as pstr, \
             tc.tile_pool(name="psp", bufs=1, space="PSUM") as psp, \
             tc.tile_pool(name="psadx", bufs=1, space="PSUM") as psadx, \
             tc.tile_pool(name="psadl", bufs=1, space="PSUM") as psadl, \
             tc.tile_pool(name="m0tp", bufs=1) as m0tp, \
             tc.tile_pool(name="dram", bufs=1, space="DRAM") as dram:

            xT   = xtp.tile([128, NPAD], F16)
            xsT  = xtp.tile([128, SHARD_PAD], F16)   # own-shard transposed output
            waug = constp.tile([128, 3, 130], F16)
            brep = constp.tile([128, 3, 128], F16)
            linw = constp.tile([128, C_OUT], F16)
            linb = constp.tile([128, C_OUT], F32)
            iota = constp.tile([128, 128], F16)
            idm  = constp.tile([128, 128], F16)
            bl   = constp.tile([128, WPC], F32)
            ilo  = constp.tile([128, n_lo // 16], I16)
            ihi  = constp.tile([128, n_hi // 16], I16)
            dstl = constp.tile([128, T_total], F16)
            nc.sync.dma_start(out=xsT[:], in_=xsT0_in[:])
            for t, s in [(xT, xT_in), (waug, waug_in), (brep, brep_in),
                         (linw, linw_in), (linb, linb_in), (iota, iota_in),
                         (idm, idm_in), (bl, bl_in), (ilo, ilo_in),
                         (ihi, ihi_in), (dstl, dstl_in)]:
                nc.sync.dma_start(out=t[:], in_=s[:])

            negshift = constp.tile([128, 1], F32)
            nc.vector.memset(negshift[:], -EXP_SHIFT)
            htab = dram.tile([NPAD, ROW_F16], F16)
            pool_bi = dram.tile([128, 129], F32)
            pool_bo = dram.tile([128, 129], F32, addr_space="Shared")

            # =========================================================
            def table_build(layer):
                BATCH = 4
                nchunks = NPAD // 128
                for b0 in range(0, nchunks, BATCH):
                    bn = min(BATCH, nchunks - b0)
                    stg = stp.tile([128, BATCH, ROW_F16], F16, tag="stg")
                    stg32 = stg[:].bitcast(F32)
                    nc.vector.memset(stg[:, :, 132:256], 0.0)
                    for j in range(bn):
                        cid = b0 + j
                        ps = pst.tile([128, 130], F32, tag="tab")
                        nc.tensor.matmul(ps[:], xT[:, cid * 128:(cid + 1) * 128],
                                         waug[:, layer, :], start=True, stop=True,
                                         skip_group_check=True)
                        if j % 2 == 0:
                            nc.vector.tensor_copy(stg[:, j, 0:128], ps[:, 0:128])
                        else:
                            nc.scalar.activation(stg[:, j, 0:128], ps[:, 0:128], AF.Copy)
                        nc.vector.tensor_copy(stg32[:, j, 64:66], ps[:, 128:130])
                    nc.sync.dma_start(
                        out=htab[b0 * 128:(b0 + bn) * 128, :]
                            .rearrange("(b p) e -> p b e", p=128),
                        in_=stg[:, 0:bn, :])

            # =========================================================
            def edge_phase(layer):
                # per-window a_dst . x_own via tiny matmuls over xsT
                adlps = psadl.tile([128, WPC], F32, tag="adl", name=f"adlps_{layer}")
                for w in range(WPC):
                    nc.tensor.matmul(adlps[:, w:w + 1], xsT[:, w * 128:(w + 1) * 128],
                                     waug[:, layer, 129:130], start=True, stop=True,
                                     skip_group_check=True)
                adl16 = ewp.tile([128, WPC], F16, tag="adl16", name=f"adl16_{layer}", bufs=1)
                nc.scalar.activation(adl16[:], adlps[:], AF.Copy)
                if dump_dbg and layer == 0:
                    nc.sync.dma_start(out=adl_out[:], in_=adl16[:])
                t0 = 0; off_lo = 0; off_hi = 0
                pool_ps = psp.tile([128, 129], F32, tag="pool", name="pool_ps") if (with_pool and layer == n_layers - 1) else None
                for ch, meta in enumerate(chunks_meta):
                    ct = sum(nt for (_, _, nt) in meta)
                    gt = gathp.tile([128, CT_MAX, ROW_F16], F16, tag="g")
                    tt = 0
                    for want_hi in (0, 1):
                        n_seg = sum(nt for (_, hi, nt) in meta if hi == want_hi) * 128
                        if n_seg == 0:
                            continue
                        src_ap = htab[LO_ROWS:NPAD, :] if want_hi else htab[0:LO_ROWS, :]
                        if want_hi:
                            idxs = ihi[:, off_hi // 16:(off_hi + n_seg) // 16]
                            off_hi += n_seg
                        else:
                            idxs = ilo[:, off_lo // 16:(off_lo + n_seg) // 16]
                            off_lo += n_seg
                        nc.gpsimd.dma_gather(
                            out_ap=gt[:, tt:tt + n_seg // 128, :], in_ap=src_ap,
                            idxs_ap=idxs, num_idxs=n_seg, num_idxs_reg=n_seg,
                            elem_size=ROW_F16, single_packet=False,
                            queue_num=nc._gq[0] % NQUEUES)
                        nc._gq[0] += 1
                        tt += n_seg // 128
                    if ch == 0 and dump_g is not False and g_out is not None:
                        nc.sync.dma_start(out=g_out[:], in_=gt[:, 0:ct, :])
                    if edge_mode < 2:
                        t0 += ct
                        continue
                    m0t = m0tp.tile([128, CT_MAX * 128], mybir.dt.float8e4, tag="m0t")
                    nc.sync.dma_start(out=m0t[:, 0:ct * 128],
                                      in_=m0t_in[:, t0 * 128:(t0 + ct) * 128])
                    adx = psadx.tile([128, 512], F32, tag="adx", name=f"adx_{layer}_{ch}")
                    # first/last tile per window (also used for expand rhs)
                    ftw = {}
                    _tt = 0
                    for (w, hi, nt) in meta:
                        for _ in range(nt):
                            ftw[_tt] = w
                            _tt += 1
                    for _tt in range(ct):
                        nc.tensor.matmul(adx[:, _tt:_tt + 1],
                                         m0t[:, _tt * 128:(_tt + 1) * 128],
                                         adl16[:, ftw[_tt]:ftw[_tt] + 1],
                                         start=True, stop=True, skip_group_check=True)
                    g32 = gt[:].bitcast(F32)
                    z  = ewp.tile([128, CT_MAX], F32, tag="z")
                    e1 = ewp.tile([128, CT_MAX], F32, tag="e1")
                    ef = ewp.tile([128, CT_MAX], F16, tag="ef")
                    nc.vector.tensor_tensor(z[:, 0:ct].unsqueeze(2),
                                            g32[:, 0:ct, 64:65], adx[:, 0:ct].unsqueeze(2), OP.add)
                    nc.scalar.activation(e1[:, 0:ct], z[:, 0:ct], AF.Exp, bias=negshift[:])
                    nc.scalar.activation(z[:, 0:ct], z[:, 0:ct], AF.Exp, bias=negshift[:], scale=0.2)
                    nc.vector.tensor_tensor(ef[:, 0:ct], e1[:, 0:ct], z[:, 0:ct], OP.max)
                    nc.vector.memset(gt[:, 0:ct, 128:130], 1.0)
                    nc.vector.tensor_tensor(
                        gt[:, 0:ct, 0:130], gt[:, 0:ct, 0:130],
                        ef[:, 0:ct].unsqueeze(2).to_broadcast((128, ct, 130)), OP.mult)
                    if edge_mode < 3:
                        t0 += ct
                        continue
                    m0 = m0p.tile([128, CT_MAX, 128], F16, tag="m0")
                    nc.vector.tensor_tensor(
                        m0[:, 0:ct, :],
                        iota[:].unsqueeze(1).to_broadcast((128, ct, 128)),
                        dstl[:, t0:t0 + ct].unsqueeze(2).to_broadcast((128, ct, 128)),
                        OP.is_equal)
                    if dump_dbg and ch == 0:
                        acp = stp.tile([128, 512], F32, tag="stg", name="acp")
                        nc.vector.memset(acp[:], 0.0)
                        nc.vector.tensor_copy(acp[:, 0:ct], adx[:, 0:ct])
                        nc.sync.dma_start(out=adx_out[:], in_=acp[:])
                        nc.sync.dma_start(out=ef_out[:], in_=ef[:, 0:ct])
                        nc.sync.dma_start(out=m0_out[:], in_=m0[:, 0:ct, :])
                    # first/last tile per window
                    ft, lt = {}, {}
                    tt = 0
                    for (w, hi, nt) in meta:
                        for _ in range(nt):
                            if w not in ft: ft[w] = tt
                            lt[w] = tt
                            tt += 1
                    psws = {w: psw.tile([128, 129], F32, tag="win", name=f"win_{layer}_{ch}_{w}") for w in ft}
                    tt = 0
                    for (w, hi, nt) in meta:
                        for _ in range(nt):
                            nc.tensor.matmul(psws[w][:], m0[:, tt, :], gt[:, tt, 0:129],
                                             start=(tt == ft[w]), stop=(tt == lt[w]),
                                             skip_group_check=True)
                            tt += 1
                    if edge_mode < 4:
                        t0 += ct
                        continue
                    for w in sorted(ft):
                        ps = psws[w]
                        if dump_dbg and ch == 0 and w == 0:
                            pcp = evp.tile([128, 129], F32, tag="pcp", name="pcp")
                            nc.vector.tensor_copy(pcp[:], ps[:])
                            nc.sync.dma_start(out=ps_out[:], in_=pcp[:])
                        dn = evp.tile([128, 1], F32, tag="dn")
                        nc.vector.tensor_scalar_max(dn[:], ps[:, 128:129], 1e-6)
                        rc = evp.tile([128, 1], F32, tag="rc")
                        nc.vector.reciprocal(rc[:], dn[:])
                        xw = evp.tile([128, 128], F16, tag="xw")
                        nc.scalar.activation(xw[:], ps[:, 0:128], AF.Copy, scale=rc[:])
                        nc.vector.tensor_tensor(xw[:], xw[:], brep[:, layer, :], OP.add)
                        nc.vector.tensor_scalar_max(xw[:], xw[:], 0.0)
                        if pool_ps is None:
                            tp = pstr.tile([128, 128], F16, tag="tr")
                            nc.tensor.transpose(tp[:], xw[:], idm[:])
                            nc.vector.tensor_copy(xsT[:, w * 128:(w + 1) * 128], tp[:])
                        else:
                            ob = evp.tile([128, 128], F16, tag="ob")
                            nc.vector.tensor_scalar(ob[:], iota[:], bl[:, w:w + 1], None,
                                                    OP.is_equal)
                            x1 = evp.tile([128, 129], F16, tag="x1")
                            nc.vector.tensor_copy(x1[:, 0:128], xw[:])
                            nc.vector.memset(x1[:, 128:129], 1.0)
                            nc.tensor.matmul(pool_ps[:], ob[:], x1[:],
                                             start=(w == 0), stop=(w == WPC - 1),
                                             skip_group_check=True)
                    t0 += ct
                return pool_ps

            # ================= main =================
            for layer in range(n_layers):
                if layer > 0:
                    bounce_in = dram.tile([128, SHARD_PAD], F16, name=f"bi_{layer}", tag=f"bi_{layer}")
                    bounce_out = dram.tile([NCORES, 128, SHARD_PAD], F16, addr_space="Shared", name=f"bo_{layer}", tag=f"bo_{layer}")
                    nc.sync.dma_start(out=bounce_in[:], in_=xsT[:])
                    nc.gpsimd.collective_compute(
                        "AllGather", OP.bypass, replica_groups=[list(range(NCORES))],
                        ins=[bounce_in[:].opt()], outs=[bounce_out[:].opt()])
                    nc.sync.dma_start(out=xT[:].rearrange("p (r c) -> p r c", r=NCORES),
                                      in_=bounce_out[:].rearrange("r p c -> p r c"))
                table_build(layer)
                pool_ps = edge_phase(layer) if do_edge else None

            if dump_htab:
                hcp = gathp.tile([128, dump_htab // 128, ROW_F16], F16, tag="g", name="hcp")
                nc.sync.dma_start(out=hcp[:], in_=htab[0:dump_htab, :].rearrange("(b p) e -> p b e", p=128))
                nc.sync.dma_start(out=htab_out[:].rearrange("(b p) e -> p b e", p=128), in_=hcp[:])
            if dump_xsT:
                nc.sync.dma_start(out=xsT_out[:], in_=xsT[:])
            if not with_pool:
                zz = evp.tile([128, C_OUT], F32, tag="res")
                nc.vector.memset(zz[:], 0.0)
                nc.sync.dma_start(out=out_t[:], in_=zz[:])
                return nc
            pooled = evp.tile([128, 129], F32, tag="pooled")
            nc.vector.tensor_copy(pooled[:], pool_ps[:])
            nc.sync.dma_start(out=pool_bi[:], in_=pooled[:])
            nc.gpsimd.collective_compute(
                "AllReduce", OP.add, replica_groups=[list(range(NCORES))],
                ins=[pool_bi[:].opt()], outs=[pool_bo[:].opt()])
            nc.sync.dma_start(out=pooled[:], in_=pool_bo[:])
            cnt = evp.tile([128, 1], F32, tag="cnt")
            nc.vector.tensor_scalar_max(cnt[:], pooled[:, 128:129], 1.0)
            rcn = evp.tile([128, 1], F32, tag="rcn")
            nc.vector.reciprocal(rcn[:], cnt[:])
            pm = evp.tile([128, 128], F16, tag="pm")
            nc.scalar.activation(pm[:], pooled[:, 0:128], AF.Copy, scale=rcn[:])
            pt = pstr.tile([128, 128], F16, tag="tr")
            nc.tensor.transpose(pt[:], pm[:], idm[:])
            pts = evp.tile([128, 128], F16, tag="pts")
            nc.vector.tensor_copy(pts[:], pt[:])
            ho = psw.tile([128, 129], F32, tag="win")
            nc.tensor.matmul(ho[:, 0:C_OUT], pts[:], linw[:], start=True, stop=True,
                             skip_group_check=True)
            res = evp.tile([128, C_OUT], F32, tag="res")
            nc.vector.tensor_tensor(res[:], ho[:, 0:C_OUT], linb[:], OP.add)
            nc.sync.dma_start(out=out_t[:], in_=res[:])
    return nc


def run(inputs, trace=False, n_layers=3, with_pool=True, dump_xsT=False, do_edge=True, dump_htab=0, edge_mode=4, dump_g=False, dump_dbg=False):
    """Full pipeline: host prep -> build -> run on 8 cores -> [G, C_OUT] f32."""
    chunks_meta, cores, T_total, n_lo, n_hi = prep_edges(np.asarray(inputs["edge_index"]))
    const_ins = make_weight_inputs(
        np.asarray(inputs["W1"]), np.asarray(inputs["a_src1"]), np.asarray(inputs["a_dst1"]), np.asarray(inputs["b1"]),
        np.asarray(inputs["W2"]), np.asarray(inputs["a_src2"]), np.asarray(inputs["a_dst2"]), np.asarray(inputs["b2"]),
        np.asarray(inputs["W3"]), np.asarray(inputs["a_src3"]), np.asarray(inputs["a_dst3"]), np.asarray(inputs["b3"]),
        np.asarray(inputs["lin_W"]), np.asarray(inputs["lin_b"]), np.asarray(inputs["x"]))
    batch = np.asarray(inputs["batch"])

    nc = bacc.Bacc("TRN2", target_bir_lowering=False, debug=False, num_devices=NCORES,
                   num_swdge_queues=NQUEUES)
    nc._gq = [0]
    build(nc, chunks_meta, T_total, n_lo, n_hi, n_layers=n_layers, with_pool=with_pool, dump_xsT=dump_xsT, do_edge=do_edge, dump_htab=dump_htab, edge_mode=edge_mode, dump_g=dump_g, dump_dbg=dump_dbg)
    nc.compile()
    split_waits(nc)

    in_maps = []
    for c in range(NCORES):
        m = dict(const_ins)
        m["batchl"] = make_batch_input(batch, c)
        m["idxlo"] = cores[c]["idxlo"]
        m["m0t"] = cores[c]["m0t"]
        m["xsT0"] = make_xsT0(np.asarray(inputs["x"]), c)
        m["idxhi"] = cores[c]["idxhi"]
        m["dstl"] = cores[c]["dstl"]
        in_maps.append(m)
    res = bass_utils.run_bass_kernel_spmd(nc, in_maps, core_ids=list(range(NCORES)),
                                          trace=trace)
    return res.results[0], res


def kernel(**inputs):
    """Harness entry: full unsharded inputs -> [128, 10] fp32 output."""
    out, _ = run(inputs)
    if isinstance(out, dict):
        out = out["out"]
    return np.asarray(out, dtype=np.float32)



# revision 26
# speedup vs baseline: 2.0324x; 1.7362x over previous
"""GAT 3-layer Bass kernel for 8 trn2 cores.

v2 design:
- Each core owns a contiguous shard of 6250 dst nodes (49 windows of 128).
- Per layer, each core computes the (rotated) table rows for its OWN nodes
  only, inside the previous layer's window-evacuation path; an AllGather
  shares the full 50176-row table (256B rows) across cores.
- Edge aggregation: per chunk (2 windows), dma_gather fetches per-edge
  256B rows from the shared table; attention weights ef are computed from
  the gathered alpha_src (rotated coord 0) plus a one-hot-matmul scatter of
  the per-window alpha_dst; a one-hot matmul accumulates the softmax
  numerator/denominator per window in PSUM.
- Rotation: T_L = diag(||a_src||,1,..) @ Q_L with Q rows 0/1 spanning
  (a_src, a_dst); table rows are h' = h @ T^T so h'[0] == alpha_src; the
  inverse R = D^{-1} Q is applied per window before relu.
- Edge padding uses trailing -1 indices which the gather ucode pops (no
  descriptor-generation cost on the GPSIMD critical path).
"""
import numpy as np
import concourse.bacc as bacc
import concourse.bass as bass
from concourse import bass_utils
from concourse.tile import TileContext
import concourse.mybir as mybir

N, H, C_OUT, G = 50000, 128, 10, 128
NCORES = 8
NPC = N // NCORES            # 6250
WPC = 49                     # 128-dst windows per core
CHUNK_W = 2
NCHUNK = (WPC + CHUNK_W - 1) // CHUNK_W   # 25
NQUEUES = 4
NGT = 3                      # gather buffer depth
PAD_IDX = 0                  # gather pad index (-1 = popped by ucode)
SHARD_PAD = WPC * 128        # 6272
NPAD = SHARD_PAD * NCORES    # 50176
ROW = 128                    # gather row: 128 f16 = 256B
LO_ROWS = 32768
EXP_SHIFT = 4.0

F16, F32, I16 = mybir.dt.float16, mybir.dt.float32, mybir.dt.int16
F8 = mybir.dt.float8e4
AF = mybir.ActivationFunctionType
OP = mybir.AluOpType


def prep_edges(edge_index):
    """Edge partition/packing. Returns (chunks_meta, per-core tensors, sizes).

    chunks_meta[ch] = dict(tg0, tg1, ct, slots=[(tile, window), ...])
    Slot list (tile-major) is uniform across cores; per-core dstl/m0t encode
    each slot's membership. Gather idx arrays carry trailing -1 padding.
    """
    src = np.concatenate([edge_index[0], np.arange(N)]).astype(np.int64)
    dst = np.concatenate([edge_index[1], np.arange(N)]).astype(np.int64)
    row_id = (src // NPC) * SHARD_PAD + (src % NPC)

    groups = {}
    for c in range(NCORES):
        m = (dst // NPC) == c
        r, dl = row_id[m], dst[m] - c * NPC
        win = dl // 128
        for ch in range(NCHUNK):
            wlo, whi = 2 * ch, min(2 * ch + 1, WPC - 1)
            inch = (win >= wlo) & (win <= whi)
            for hi in (0, 1):
                mm = inch & ((r >= LO_ROWS) == bool(hi))
                rr, ww, dd = r[mm], win[mm], dl[mm]
                o = np.lexsort((rr, ww))
                groups[(c, ch, hi)] = (rr[o] - (LO_ROWS if hi else 0),
                                       ww[o], (dd - ww * 128)[o])

    chunks = []
    idx_arr = {0: [[] for _ in range(NCORES)], 1: [[] for _ in range(NCORES)]}
    dstl_cols = [[] for _ in range(NCORES)]
    m0t_blocks = [[] for _ in range(NCORES)]
    for ch in range(NCHUNK):
        tg = {}
        for hi in (0, 1):
            mx = max(len(groups[(c, ch, hi)][0]) for c in range(NCORES))
            tg[hi] = max(1, -(-mx // 128))
        slots = []
        for hi in (0, 1):
            base_t = 0 if hi == 0 else tg[0]
            for tl in range(tg[hi]):
                wset = set()
                for c in range(NCORES):
                    ww = groups[(c, ch, hi)][1][tl * 128:(tl + 1) * 128]
                    wset.update(np.unique(ww).tolist())
                if not wset:
                    wset = {2 * ch}
                for w in sorted(wset):
                    slots.append((base_t + tl, w))
        for c in range(NCORES):
            for hi in (0, 1):
                rr = groups[(c, ch, hi)][0]
                L = tg[hi] * 128
                ridx = np.full(L, PAD_IDX, np.int64)
                ridx[:len(rr)] = rr
                idx_arr[hi][c].append(ridx)
            for (t, w) in slots:
                hi = 0 if t < tg[0] else 1
                tl = t if hi == 0 else t - tg[0]
                _, ww, dd = groups[(c, ch, hi)]
                wt = ww[tl * 128:(tl + 1) * 128]
                dt_ = dd[tl * 128:(tl + 1) * 128]
                dcol = np.full(128, -1.0, np.float32)
                sel = np.where(wt == w)[0]
                dcol[sel] = dt_[sel]
                dstl_cols[c].append(dcol)
                m0t_blocks[c].append(
                    (np.arange(128)[:, None] == dcol[None, :]))
        chunks.append(dict(tg0=tg[0], tg1=tg[1], ct=tg[0] + tg[1], slots=slots))

    def wrap16(a):
        a = a.astype(np.int16).reshape(-1, 16).T
        return np.tile(a, (8, 1)).copy()

    cores = []
    for c in range(NCORES):
        lo = np.concatenate(idx_arr[0][c])
        hi = np.concatenate(idx_arr[1][c])
        dstl = np.stack(dstl_cols[c], axis=1).astype(np.float16)  # [128, NSLOT]
        m0t = np.concatenate(m0t_blocks[c], axis=1).astype(
            mybir.dt.np(F8))                                      # [128, NSLOT*128]
        cores.append(dict(idxlo=wrap16(lo), idxhi=wrap16(hi),
                          dstl=dstl, m0t=m0t))
    n_lo = sum(len(a) for a in idx_arr[0][0])
    n_hi = sum(len(a) for a in idx_arr[1][0])
    NSLOT = sum(len(ch["slots"]) for ch in chunks)
    return chunks, cores, NSLOT, n_lo, n_hi


def make_weight_inputs(Ws, asrcs, adsts, bs, lin_W, lin_b):
    waug = np.zeros((128, 3, 129), np.float16)
    runr = np.zeros((128, 3, 128), np.float16)
    bcol = np.zeros((128, 3), np.float32)
    for i in range(3):
        W = Ws[i].astype(np.float64)
        a_s = asrcs[i].astype(np.float64)
        a_d = adsts[i].astype(np.float64)
        d0 = np.linalg.norm(a_s)
        q0 = a_s / d0
        v = a_d - (a_d @ q0) * q0
        q1 = v / np.linalg.norm(v)
        Mstack = np.column_stack([q0, q1, np.eye(128)[:, :126]])
        Qf, _ = np.linalg.qr(Mstack)
        if Qf[:, 0] @ q0 < 0:
            Qf[:, 0] *= -1
        if Qf[:, 1] @ q1 < 0:
            Qf[:, 1] *= -1
        Qr = Qf.T                      # rows orthonormal; row0=q0, row1=q1
        T = Qr.copy(); T[0] *= d0      # h' = h @ T^T ; h'[0] = alpha_src
        R = Qr.copy(); R[0] /= d0      # h = h' @ R
        assert np.allclose(T.T @ R, np.eye(128), atol=1e-10)
        waug[:, i, 0:128] = W @ T.T
        waug[:, i, 128] = W @ a_d
        runr[:, i, :] = R
        bcol[:, i] = bs[i]
    iota = np.broadcast_to(np.arange(128, dtype=np.float16), (128, 128)).copy()
    return dict(
        waug=waug, runr=runr, bcol=bcol,
        linw=lin_W.astype(np.float16),
        linb=np.broadcast_to(lin_b.astype(np.float32), (128, C_OUT)).copy(),
        iota=iota, idm=np.eye(128, dtype=np.float16),
    )


def make_xsT0(x, core):
    out = np.zeros((128, SHARD_PAD), np.float16)
    out[:, :NPC] = x[core * NPC:(core + 1) * NPC].astype(np.float16).T
    return out


def make_batch_input(batch, core):
    bl = np.full((128, WPC), -1.0, np.float32)
    ids = batch[core * NPC:(core + 1) * NPC].astype(np.float32)
    for w in range(WPC):
        seg = ids[w * 128:(w + 1) * 128]
        bl[: len(seg), w] = seg
    return bl


def split_waits(nc, maxw=1):
    n = 0
    for func in nc.m.functions:
        for block in func.blocks:
            new = []
            for inst in block.instructions:
                si = inst.sync_info
                if si is not None and si.on_wait and len(si.on_wait) > maxw:
                    w = list(si.on_wait); extra, keep = w[:-maxw], w[-maxw:]
                    while extra:
                        ck, extra = extra[:maxw], extra[maxw:]
                        new.append(mybir.InstNoOp(name=f"ws-{n}", engine=inst.engine,
                            sync_info=mybir.SyncInfo(on_wait=ck, on_update=[])))
                        n += 1
                    si.on_wait = keep
                new.append(inst)
            block.instructions = new
    return n


def build(nc, chunks, NSLOT, n_lo, n_hi, n_layers=3, edge_mode=3,
          with_pool=True, dump_htab=0):
    CT_MAX = max(c["ct"] for c in chunks)
    NS_MAX = max(len(c["slots"]) for c in chunks)

    waug_in = nc.dram_tensor("waug", [128, 3, 129], F16, kind="ExternalInput")
    runr_in = nc.dram_tensor("runr", [128, 3, 128], F16, kind="ExternalInput")
    bcol_in = nc.dram_tensor("bcol", [128, 3], F32, kind="ExternalInput")
    linw_in = nc.dram_tensor("linw", [128, C_OUT], F16, kind="ExternalInput")
    linb_in = nc.dram_tensor("linb", [128, C_OUT], F32, kind="ExternalInput")
    iota_in = nc.dram_tensor("iota", [128, 128], F16, kind="ExternalInput")
    idm_in  = nc.dram_tensor("idm", [128, 128], F16, kind="ExternalInput")
    bl_in   = nc.dram_tensor("batchl", [128, WPC], F32, kind="ExternalInput")
    ilo_in  = nc.dram_tensor("idxlo", [128, n_lo // 16], I16, kind="ExternalInput")
    ihi_in  = nc.dram_tensor("idxhi", [128, n_hi // 16], I16, kind="ExternalInput")
    dstl_in = nc.dram_tensor("dstl", [128, NSLOT], F16, kind="ExternalInput")
    m0t_in  = nc.dram_tensor("m0t", [128, NSLOT * 128], F8, kind="ExternalInput")
    xsT0_in = nc.dram_tensor("xsT0", [128, SHARD_PAD], F16, kind="ExternalInput")
    out_t   = nc.dram_tensor("out", [G, C_OUT], F32, kind="ExternalOutput")
    htab_out = (nc.dram_tensor("htab_out", [dump_htab, ROW], F16,
                               kind="ExternalOutput") if dump_htab else None)

    with TileContext(nc) as tc:
        with tc.tile_pool(name="const", bufs=1) as constp, \
             tc.tile_pool(name="gath", bufs=1) as gathp, \
             tc.tile_pool(name="m0p", bufs=2) as m0p, \
             tc.tile_pool(name="rhsp", bufs=2) as rhsp, \
             tc.tile_pool(name="ewp", bufs=2) as ewp, \
             tc.tile_pool(name="evac", bufs=3) as evp, \
             tc.tile_pool(name="stage", bufs=3) as stp, \
             tc.tile_pool(name="m0tp", bufs=2) as m0tp, \
             tc.tile_pool(name="psw", bufs=3, space="PSUM") as psw, \
             tc.tile_pool(name="psadx", bufs=1, space="PSUM") as psadx, \
             tc.tile_pool(name="pstr", bufs=1, space="PSUM") as pstr, \
             tc.tile_pool(name="psunx", bufs=2, space="PSUM") as psunx, \
             tc.tile_pool(name="psp", bufs=1, space="PSUM") as psp, \
             tc.tile_pool(name="dram", bufs=1, space="DRAM") as dram:

            waug = constp.tile([128, 3, 129], F16)
            runr = constp.tile([128, 3, 128], F16)
            bcol = constp.tile([128, 3], F32)
            linw = constp.tile([128, C_OUT], F16)
            linb = constp.tile([128, C_OUT], F32)
            iota = constp.tile([128, 128], F16)
            idm  = constp.tile([128, 128], F16)
            bl   = constp.tile([128, WPC], F32)
            ilo  = constp.tile([128, n_lo // 16], I16)
            ihi  = constp.tile([128, n_hi // 16], I16)
            dstl = constp.tile([128, NSLOT], F16)
            xsT0 = constp.tile([128, SHARD_PAD], F16)
            for t, s in [(waug, waug_in), (runr, runr_in), (bcol, bcol_in),
                         (linw, linw_in), (linb, linb_in), (iota, iota_in),
                         (idm, idm_in), (bl, bl_in), (ilo, ilo_in),
                         (ihi, ihi_in), (dstl, dstl_in), (xsT0, xsT0_in)]:
                nc.sync.dma_start(out=t[:], in_=s[:])

            negshift = constp.tile([128, 1], F32)
            nc.vector.memset(negshift[:], -EXP_SHIFT)
            adl = [constp.tile([128, WPC], F16, name=f"adl{i}") for i in range(2)]

            gt_bufs = [gathp.tile([128, CT_MAX, ROW], F16, name=f"gt{i}",
                                  tag=f"gt{i}") for i in range(NGT)]
            for gtb in gt_bufs:
                nc.vector.memset(gtb[:], 0.0)

            htab_own = [dram.tile([SHARD_PAD, ROW], F16, name=f"hown{i}",
                                  tag=f"hown{i}") for i in range(2)]
            htab_sh = [dram.tile([NCORES, SHARD_PAD, ROW], F16,
                                 addr_space="Shared", name=f"hsh{i}",
                                 tag=f"hsh{i}") for i in range(3)]
            pool_bi = dram.tile([128, 129], F32)
            pool_bo = dram.tile([128, 129], F32, addr_space="Shared")

            # ---- produce own-shard table rows for table `ti` from y [f, n] ----
            def own_rows(ti, w, y_ap):
                ps = psunx.tile([128, 129], F32, tag="unx")
                nc.tensor.matmul(ps[:], y_ap, waug[:, ti, :], start=True,
                                 stop=True, skip_group_check=True)
                st = stp.tile([128, 128], F16, tag="st")
                nc.scalar.activation(st[:], ps[:, 0:128], AF.Copy)
                nc.vector.tensor_copy(adl[ti % 2][:, w:w + 1], ps[:, 128:129])
                nc.sync.dma_start(
                    out=htab_own[ti % 2][w * 128:(w + 1) * 128, :]
                        .rearrange("(b p) e -> p b e", p=128),
                    in_=st[:].unsqueeze(1))

            htab_loc = [dram.tile([NCORES, SHARD_PAD, ROW], F16,
                                  name=f"hloc{i}", tag=f"hloc{i}")
                        for i in range(2)]

            def allgather(ti):
                nc.gpsimd.collective_compute(
                    "AllGather", OP.bypass, replica_groups=[list(range(NCORES))],
                    ins=[htab_own[ti % 2][:].opt()],
                    outs=[htab_sh[ti][:].opt()])
                nc.sync.dma_start(out=htab_loc[ti % 2][:], in_=htab_sh[ti][:])

            pool_ps = psp.tile([128, 129], F32, tag="pool", name="pool_ps")

            # ---- per-window output path for edge layer `layer` ----
            def window_out(layer, w, ps):
                dn = evp.tile([128, 1], F32, tag="dn")
                nc.vector.tensor_scalar_max(dn[:], ps[:, 128:129], 1e-6)
                rc = evp.tile([128, 1], F32, tag="rc")
                nc.vector.reciprocal(rc[:], dn[:])
                xw = evp.tile([128, 128], F16, tag="xw")
                nc.scalar.activation(xw[:], ps[:, 0:128], AF.Copy, scale=rc[:])
                tp = pstr.tile([128, 128], F16, tag="tr")
                nc.tensor.transpose(tp[:], xw[:], idm[:])
                xwT = evp.tile([128, 128], F16, tag="xwT")
                nc.vector.tensor_copy(xwT[:], tp[:])
                up = psunx.tile([128, 129], F32, tag="unx", name=f"u_{layer}_{w}")
                nc.tensor.matmul(up[:, 0:128], runr[:, layer, :], xwT[:],
                                 start=True, stop=True, skip_group_check=True)
                y = evp.tile([128, 128], F16, tag="y")
                nc.scalar.activation(y[:], up[:, 0:128], AF.Relu,
                                     bias=bcol[:, layer:layer + 1])
                if layer < 2:
                    own_rows(layer + 1, w, y[:])
                else:
                    tp2 = pstr.tile([128, 128], F16, tag="tr",
                                    name=f"tr2_{w}")
                    nc.tensor.transpose(tp2[:], y[:], idm[:])
                    x1 = evp.tile([128, 129], F16, tag="x1")
                    nc.vector.tensor_copy(x1[:, 0:128], tp2[:])
                    nc.vector.memset(x1[:, 128:129], 1.0)
                    ob = evp.tile([128, 128], F16, tag="ob")
                    nc.vector.tensor_scalar(ob[:], iota[:], bl[:, w:w + 1],
                                            None, OP.is_equal)
                    nc.tensor.matmul(pool_ps[:], ob[:], x1[:],
                                     start=(w == 0), stop=(w == WPC - 1),
                                     skip_group_check=True)

            # ---- edge phase ----
            def edge_phase(layer):
                adl_cur = adl[layer % 2]
                off = {0: 0, 1: 0}
                slot_off = 0
                src_flat = htab_loc[layer % 2][:].rearrange("r s e -> (r s) e")
                for ch, meta in enumerate(chunks):
                    ct, tg0, tg1 = meta["ct"], meta["tg0"], meta["tg1"]
                    slots = meta["slots"]; ns = len(slots)
                    gt = gt_bufs[(layer * NCHUNK + ch) % NGT]
                    t0 = 0
                    for hi, tg in ((0, tg0), (1, tg1)):
                        n_seg = tg * 128
                        src_ap = (src_flat[LO_ROWS:NPAD, :] if hi
                                  else src_flat[0:LO_ROWS, :])
                        if hi:
                            idxs = ihi[:, off[1] // 16:(off[1] + n_seg) // 16]
                            off[1] += n_seg
                        else:
                            idxs = ilo[:, off[0] // 16:(off[0] + n_seg) // 16]
                            off[0] += n_seg
                        nc.gpsimd.dma_gather(
                            out_ap=gt[:, t0:t0 + tg, :], in_ap=src_ap,
                            idxs_ap=idxs, num_idxs=n_seg, num_idxs_reg=n_seg,
                            elem_size=ROW, single_packet=False,
                            queue_num=nc._gq[0] % NQUEUES)
                        nc._gq[0] += 1
                        t0 += tg
                    if edge_mode < 1:
                        slot_off += ns
                        continue
                    m0t = m0tp.tile([128, NS_MAX * 128], F8, tag="m0t")
                    nc.sync.dma_start(
                        out=m0t[:, 0:ns * 128],
                        in_=m0t_in[:, slot_off * 128:(slot_off + ns) * 128])
                    adx = psadx.tile([128, CT_MAX], F32, tag="adx")
                    tile_slots = {}
                    for si, (t, w) in enumerate(slots):
                        tile_slots.setdefault(t, []).append(si)
                    for t, sis in tile_slots.items():
                        for k, si in enumerate(sis):
                            _, w = slots[si]
                            nc.tensor.matmul(
                                adx[:, t:t + 1], m0t[:, si * 128:(si + 1) * 128],
                                adl_cur[:, w:w + 1], start=(k == 0),
                                stop=(k == len(sis) - 1), skip_group_check=True)
                    z  = ewp.tile([128, CT_MAX], F32, tag="z")
                    e1 = ewp.tile([128, CT_MAX], F32, tag="e1")
                    ef = ewp.tile([128, CT_MAX], F16, tag="ef")
                    nc.vector.tensor_tensor(z[:, 0:ct].unsqueeze(2),
                                            gt[:, 0:ct, 0:1],
                                            adx[:, 0:ct].unsqueeze(2), OP.add)
                    nc.scalar.activation(e1[:, 0:ct], z[:, 0:ct], AF.Exp,
                                         bias=negshift[:])
                    nc.scalar.activation(z[:, 0:ct], z[:, 0:ct], AF.Exp,
                                         bias=negshift[:], scale=0.2)
                    nc.vector.tensor_tensor(ef[:, 0:ct], e1[:, 0:ct],
                                            z[:, 0:ct], OP.max)
                    rhs = rhsp.tile([128, CT_MAX, 130], F16, tag="rhs")
                    nc.vector.tensor_tensor(
                        rhs[:, 0:ct, 0:128], gt[:, 0:ct, :],
                        ef[:, 0:ct].unsqueeze(2).to_broadcast((128, ct, 128)),
                        OP.mult)
                    nc.vector.tensor_copy(rhs[:, 0:ct, 128:129],
                                          ef[:, 0:ct].unsqueeze(2))
                    if edge_mode < 2:
                        slot_off += ns
                        continue
                    m0 = m0p.tile([128, NS_MAX, 128], F16, tag="m0")
                    nc.vector.tensor_tensor(
                        m0[:, 0:ns, :],
                        iota[:].unsqueeze(1).to_broadcast((128, ns, 128)),
                        dstl[:, slot_off:slot_off + ns].unsqueeze(2)
                            .to_broadcast((128, ns, 128)),
                        OP.is_equal)
                    wf, wl = {}, {}
                    for si, (t, w) in enumerate(slots):
                        if w not in wf:
                            wf[w] = si
                        wl[w] = si
                    psws = {w: psw.tile([128, 129], F32, tag="win",
                                        name=f"win{layer}_{ch}_{w}")
                            for w in wf}
                    for si, (t, w) in enumerate(slots):
                        nc.tensor.matmul(psws[w][:], m0[:, si, :],
                                         rhs[:, t, 0:129],
                                         start=(si == wf[w]),
                                         stop=(si == wl[w]),
                                         skip_group_check=True)
                    if edge_mode >= 3:
                        for w in sorted(wf):
                            window_out(layer, w, psws[w])
                    slot_off += ns

            # ================= main =================
            for w in range(WPC):
                own_rows(0, w, xsT0[:, w * 128:(w + 1) * 128])
            allgather(0)
            for layer in range(n_layers):
                edge_phase(layer)
                if layer < 2 and edge_mode >= 3:
                    allgather(layer + 1)
            if dump_htab:
                hcp = stp.tile([128, dump_htab // 128, ROW], F16, tag="hcp")
                nc.sync.dma_start(
                    out=hcp[:],
                    in_=htab_loc[0][:].rearrange("r s e -> (r s) e")
                        [0:dump_htab, :].rearrange("(b p) e -> p b e", p=128))
                nc.sync.dma_start(
                    out=htab_out[:].rearrange("(b p) e -> p b e", p=128),
                    in_=hcp[:])
            if not with_pool or n_layers < 3 or edge_mode < 3:
                zz = evp.tile([128, C_OUT], F32, tag="res")
                nc.vector.memset(zz[:], 0.0)
                nc.sync.dma_start(out=out_t[:], in_=zz[:])
                return nc

            # ---- pool + final linear ----
            pooled = evp.tile([128, 129], F32, tag="pooled")
            nc.vector.tensor_copy(pooled[:], pool_ps[:])
            nc.sync.dma_start(out=pool_bi[:], in_=pooled[:])
            nc.gpsimd.collective_compute(
                "AllReduce", OP.add, replica_groups=[list(range(NCORES))],
                ins=[pool_bi[:].opt()], outs=[pool_bo[:].opt()])
            nc.sync.dma_start(out=pooled[:], in_=pool_bo[:])
            cnt = evp.tile([128, 1], F32, tag="cnt")
            nc.vector.tensor_scalar_max(cnt[:], pooled[:, 128:129], 1.0)
            rcn = evp.tile([128, 1], F32, tag="rcn")
            nc.vector.reciprocal(rcn[:], cnt[:])
            pm = evp.tile([128, 128], F16, tag="pm")
            nc.scalar.activation(pm[:], pooled[:, 0:128], AF.Copy, scale=rcn[:])
            pt = pstr.tile([128, 128], F16, tag="tr")
            nc.tensor.transpose(pt[:], pm[:], idm[:])
            pts = evp.tile([128, 128], F16, tag="pts")
            nc.vector.tensor_copy(pts[:], pt[:])
            ho = psw.tile([128, 129], F32, tag="win", name="ho")
            nc.tensor.matmul(ho[:, 0:C_OUT], pts[:], linw[:], start=True,
                             stop=True, skip_group_check=True)
            res = evp.tile([128, C_OUT], F32, tag="res")
            nc.vector.tensor_tensor(res[:], ho[:, 0:C_OUT], linb[:], OP.add)
            nc.sync.dma_start(out=out_t[:], in_=res[:])
    return nc


def run(inputs, trace=False, **build_kw):
    chunks, cores, NSLOT, n_lo, n_hi = prep_edges(np.asarray(inputs["edge_index"]))
    const_ins = make_weight_inputs(
        [np.asarray(inputs[f"W{i}"]) for i in (1, 2, 3)],
        [np.asarray(inputs[f"a_src{i}"]) for i in (1, 2, 3)],
        [np.asarray(inputs[f"a_dst{i}"]) for i in (1, 2, 3)],
        [np.asarray(inputs[f"b{i}"]) for i in (1, 2, 3)],
        np.asarray(inputs["lin_W"]), np.asarray(inputs["lin_b"]))
    batch = np.asarray(inputs["batch"])
    x = np.asarray(inputs["x"])

    nc = bacc.Bacc("TRN2", target_bir_lowering=False, debug=False,
                   num_devices=NCORES, num_swdge_queues=NQUEUES)
    nc._gq = [0]
    build(nc, chunks, NSLOT, n_lo, n_hi, **build_kw)
    nc.compile()
    split_waits(nc)

    in_maps = []
    for c in range(NCORES):
        m = dict(const_ins)
        m["batchl"] = make_batch_input(batch, c)
        m["xsT0"] = make_xsT0(x, c)
        m["idxlo"] = cores[c]["idxlo"]
        m["idxhi"] = cores[c]["idxhi"]
        m["dstl"] = cores[c]["dstl"]
        m["m0t"] = cores[c]["m0t"]
        in_maps.append(m)
    res = bass_utils.run_bass_kernel_spmd(nc, in_maps,
                                          core_ids=list(range(NCORES)),
                                          trace=trace)
    return res.results[0], res


def kernel(**inputs):
    """Harness entry: full unsharded inputs -> [128, 10] fp32 output."""
    out, _ = run(inputs)
    if isinstance(out, dict):
        out = out["out"]
    return np.asarray(out, dtype=np.float32)


# revision 47
# speedup vs baseline: 2.3718x; 1.1670x over previous
"""GAT 3-layer Bass kernel for 8 trn2 cores.

v2 design:
- Each core owns a contiguous shard of 6250 dst nodes (49 windows of 128).
- Per layer, each core computes the (rotated) table rows for its OWN nodes
  only, inside the previous layer's window-evacuation path; an AllGather
  shares the full 50176-row table (256B rows) across cores.
- Edge aggregation: per chunk (2 windows), dma_gather fetches per-edge
  256B rows from the shared table; attention weights ef are computed from
  the gathered alpha_src (rotated coord 0) plus a one-hot-matmul scatter of
  the per-window alpha_dst; a one-hot matmul accumulates the softmax
  numerator/denominator per window in PSUM.
- Rotation: T_L = diag(||a_src||,1,..) @ Q_L with Q rows 0/1 spanning
  (a_src, a_dst); table rows are h' = h @ T^T so h'[0] == alpha_src; the
  inverse R = D^{-1} Q is applied per window before relu.
- Edge padding uses trailing -1 indices which the gather ucode pops (no
  descriptor-generation cost on the GPSIMD critical path).
"""
import numpy as np
import concourse.bacc as bacc
import concourse.bass as bass
from concourse import bass_utils
from concourse.tile import TileContext
import concourse.mybir as mybir

N, H, C_OUT, G = 50000, 128, 10, 128
NCORES = 8
NPC = N // NCORES            # 6250
WPC = 49                     # 128-dst windows per core
CHUNK_W = 2
NCHUNK = (WPC + CHUNK_W - 1) // CHUNK_W   # 25
NQUEUES = 4
NGT = 3                      # gather buffer depth
PAD_IDX = 0                  # gather pad index (-1 = popped by ucode)
SHARD_PAD = WPC * 128        # 6272
NPAD = SHARD_PAD * NCORES    # 50176
ROW = 128                    # gather row: 128 f16 = 256B
LO_ROWS = 32768
EXP_SHIFT = 4.0

F16, F32, I16 = mybir.dt.float16, mybir.dt.float32, mybir.dt.int16
F8 = mybir.dt.float8e4
AF = mybir.ActivationFunctionType
OP = mybir.AluOpType


def prep_edges(edge_index):
    """Edge partition/packing. Returns (chunks_meta, per-core tensors, sizes).

    chunks_meta[ch] = dict(tg0, tg1, ct, slots=[(tile, window), ...])
    Slot list (tile-major) is uniform across cores; per-core dstl/m0t encode
    each slot's membership. Gather idx arrays carry trailing -1 padding.
    """
    src = np.concatenate([edge_index[0], np.arange(N)]).astype(np.int64)
    dst = np.concatenate([edge_index[1], np.arange(N)]).astype(np.int64)
    row_id = (src // NPC) * SHARD_PAD + (src % NPC)

    groups = {}
    for c in range(NCORES):
        m = (dst // NPC) == c
        r, dl = row_id[m], dst[m] - c * NPC
        win = dl // 128
        for ch in range(NCHUNK):
            wlo, whi = 2 * ch, min(2 * ch + 1, WPC - 1)
            inch = (win >= wlo) & (win <= whi)
            for hi in (0, 1):
                mm = inch & ((r >= LO_ROWS) == bool(hi))
                rr, ww, dd = r[mm], win[mm], dl[mm]
                o = np.lexsort((rr, ww))
                groups[(c, ch, hi)] = (rr[o] - (LO_ROWS if hi else 0),
                                       ww[o], (dd - ww * 128)[o])

    chunks = []
    idx_arr = {0: [[] for _ in range(NCORES)], 1: [[] for _ in range(NCORES)]}
    dstl_cols = [[] for _ in range(NCORES)]
    m0t_blocks = [[] for _ in range(NCORES)]
    m0f_blocks = [[] for _ in range(NCORES)]
    for ch in range(NCHUNK):
        tg = {}
        for hi in (0, 1):
            mx = max(len(groups[(c, ch, hi)][0]) for c in range(NCORES))
            tg[hi] = max(1, -(-mx // 128))
        slots = []
        for hi in (0, 1):
            base_t = 0 if hi == 0 else tg[0]
            for tl in range(tg[hi]):
                wset = set()
                for c in range(NCORES):
                    ww = groups[(c, ch, hi)][1][tl * 128:(tl + 1) * 128]
                    wset.update(np.unique(ww).tolist())
                if not wset:
                    wset = {2 * ch}
                for w in sorted(wset):
                    slots.append((base_t + tl, w))
        for c in range(NCORES):
            for hi in (0, 1):
                rr = groups[(c, ch, hi)][0]
                L = tg[hi] * 128
                ridx = np.full(L, PAD_IDX, np.int64)
                ridx[:len(rr)] = rr
                idx_arr[hi][c].append(ridx)
            for (t, w) in slots:
                hi = 0 if t < tg[0] else 1
                tl = t if hi == 0 else t - tg[0]
                _, ww, dd = groups[(c, ch, hi)]
                wt = ww[tl * 128:(tl + 1) * 128]
                dt_ = dd[tl * 128:(tl + 1) * 128]
                dcol = np.full(128, -1.0, np.float32)
                sel = np.where(wt == w)[0]
                dcol[sel] = dt_[sel]
                dstl_cols[c].append(dcol)
                m0t_blocks[c].append(
                    (np.arange(128)[:, None] == dcol[None, :]))
                m0f_blocks[c].append(
                    (dcol[:, None] == np.arange(128)[None, :]))
        chunks.append(dict(tg0=tg[0], tg1=tg[1], ct=tg[0] + tg[1], slots=slots))

    def wrap16(a):
        a = a.astype(np.int16).reshape(-1, 16).T
        return np.tile(a, (8, 1)).copy()

    cores = []
    for c in range(NCORES):
        lo = np.concatenate(idx_arr[0][c])
        hi = np.concatenate(idx_arr[1][c])
        dstl = np.stack(dstl_cols[c], axis=1).astype(np.float16)  # [128, NSLOT]
        m0t = np.concatenate(m0t_blocks[c], axis=1).astype(
            mybir.dt.np(F8))                                      # [128, NSLOT*128]
        m0f = np.concatenate(m0f_blocks[c], axis=1).astype(
            mybir.dt.np(F8))                                      # [128, NSLOT*128]
        cores.append(dict(idxlo=wrap16(lo), idxhi=wrap16(hi),
                          dstl=dstl, m0t=m0t, m0f=m0f))
    n_lo = sum(len(a) for a in idx_arr[0][0])
    n_hi = sum(len(a) for a in idx_arr[1][0])
    NSLOT = sum(len(ch["slots"]) for ch in chunks)
    return chunks, cores, NSLOT, n_lo, n_hi


def make_weight_inputs(Ws, asrcs, adsts, bs, lin_W, lin_b):
    waug = np.zeros((128, 3, 129), np.float16)
    runr = np.zeros((128, 3, 128), np.float16)
    bcol = np.zeros((128, 3), np.float32)
    for i in range(3):
        W = Ws[i].astype(np.float64)
        a_s = asrcs[i].astype(np.float64)
        a_d = adsts[i].astype(np.float64)
        d0 = np.linalg.norm(a_s)
        q0 = a_s / d0
        v = a_d - (a_d @ q0) * q0
        q1 = v / np.linalg.norm(v)
        Mstack = np.column_stack([q0, q1, np.eye(128)[:, :126]])
        Qf, _ = np.linalg.qr(Mstack)
        if Qf[:, 0] @ q0 < 0:
            Qf[:, 0] *= -1
        if Qf[:, 1] @ q1 < 0:
            Qf[:, 1] *= -1
        Qr = Qf.T                      # rows orthonormal; row0=q0, row1=q1
        T = Qr.copy(); T[0] *= d0      # h' = h @ T^T ; h'[0] = alpha_src
        R = Qr.copy(); R[0] /= d0      # h = h' @ R
        assert np.allclose(T.T @ R, np.eye(128), atol=1e-10)
        waug[:, i, 0:128] = W @ T.T
        waug[:, i, 128] = W @ a_d
        runr[:, i, :] = R
        bcol[:, i] = bs[i]
    iota = np.broadcast_to(np.arange(128, dtype=np.float16), (128, 128)).copy()
    return dict(
        waug=waug, runr=runr, bcol=bcol,
        linw=lin_W.astype(np.float16),
        linb=np.broadcast_to(lin_b.astype(np.float32), (128, C_OUT)).copy(),
        iota=iota, idm=np.eye(128, dtype=np.float16),
    )


def make_xsT0(x, core):
    out = np.zeros((128, SHARD_PAD), np.float16)
    out[:, :NPC] = x[core * NPC:(core + 1) * NPC].astype(np.float16).T
    return out


def make_batch_input(batch, core):
    bl = np.full((128, WPC), -1.0, np.float32)
    ids = batch[core * NPC:(core + 1) * NPC].astype(np.float32)
    for w in range(WPC):
        seg = ids[w * 128:(w + 1) * 128]
        bl[: len(seg), w] = seg
    return bl


def split_waits(nc, maxw=1):
    n = 0
    for func in nc.m.functions:
        for block in func.blocks:
            new = []
            for inst in block.instructions:
                si = inst.sync_info
                if si is not None and si.on_wait and len(si.on_wait) > maxw:
                    w = list(si.on_wait); extra, keep = w[:-maxw], w[-maxw:]
                    while extra:
                        ck, extra = extra[:maxw], extra[maxw:]
                        new.append(mybir.InstNoOp(name=f"ws-{n}", engine=inst.engine,
                            sync_info=mybir.SyncInfo(on_wait=ck, on_update=[])))
                        n += 1
                    si.on_wait = keep
                new.append(inst)
            block.instructions = new
    return n


def build(nc, chunks, NSLOT, n_lo, n_hi, n_layers=3, edge_mode=3,
          with_pool=True, dump_htab=0):
    CT_MAX = max(c["ct"] for c in chunks)
    NS_MAX = max(len(c["slots"]) for c in chunks)

    waug_in = nc.dram_tensor("waug", [128, 3, 129], F16, kind="ExternalInput")
    runr_in = nc.dram_tensor("runr", [128, 3, 128], F16, kind="ExternalInput")
    bcol_in = nc.dram_tensor("bcol", [128, 3], F32, kind="ExternalInput")
    linw_in = nc.dram_tensor("linw", [128, C_OUT], F16, kind="ExternalInput")
    linb_in = nc.dram_tensor("linb", [128, C_OUT], F32, kind="ExternalInput")
    iota_in = nc.dram_tensor("iota", [128, 128], F16, kind="ExternalInput")
    idm_in  = nc.dram_tensor("idm", [128, 128], F16, kind="ExternalInput")
    bl_in   = nc.dram_tensor("batchl", [128, WPC], F32, kind="ExternalInput")
    ilo_in  = nc.dram_tensor("idxlo", [128, n_lo // 16], I16, kind="ExternalInput")
    ihi_in  = nc.dram_tensor("idxhi", [128, n_hi // 16], I16, kind="ExternalInput")
    dstl_in = nc.dram_tensor("dstl", [128, NSLOT], F16, kind="ExternalInput")
    m0t_in  = nc.dram_tensor("m0t", [128, NSLOT * 128], F8, kind="ExternalInput")
    m0f_in  = nc.dram_tensor("m0f", [128, NSLOT * 128], F8, kind="ExternalInput")
    xsT0_in = nc.dram_tensor("xsT0", [128, SHARD_PAD], F16, kind="ExternalInput")
    out_t   = nc.dram_tensor("out", [G, C_OUT], F32, kind="ExternalOutput")
    htab_out = (nc.dram_tensor("htab_out", [dump_htab, ROW], F16,
                               kind="ExternalOutput") if dump_htab else None)

    with TileContext(nc) as tc:
        with tc.tile_pool(name="const", bufs=1) as constp, \
             tc.tile_pool(name="gath", bufs=1) as gathp, \
             tc.tile_pool(name="m0fp", bufs=2) as m0fp, \
             tc.tile_pool(name="rhsp", bufs=2) as rhsp, \
             tc.tile_pool(name="ewp", bufs=2) as ewp, \
             tc.tile_pool(name="evac", bufs=3) as evp, \
             tc.tile_pool(name="stage", bufs=3) as stp, \
             tc.tile_pool(name="m0tp", bufs=2) as m0tp, \
             tc.tile_pool(name="psw", bufs=3, space="PSUM") as psw, \
             tc.tile_pool(name="psadx", bufs=1, space="PSUM") as psadx, \
             tc.tile_pool(name="pstr", bufs=1, space="PSUM") as pstr, \
             tc.tile_pool(name="psunx", bufs=2, space="PSUM") as psunx, \
             tc.tile_pool(name="psp", bufs=1, space="PSUM") as psp, \
             tc.tile_pool(name="dram", bufs=1, space="DRAM") as dram:

            waug = constp.tile([128, 3, 129], F16)
            runr = constp.tile([128, 3, 128], F16)
            bcol = constp.tile([128, 3], F32)
            linw = constp.tile([128, C_OUT], F16)
            linb = constp.tile([128, C_OUT], F32)
            iota = constp.tile([128, 128], F16)
            idm  = constp.tile([128, 128], F16)
            bl   = constp.tile([128, WPC], F32)
            ilo  = constp.tile([128, n_lo // 16], I16)
            ihi  = constp.tile([128, n_hi // 16], I16)
            dstl = constp.tile([128, NSLOT], F16)
            xsT0 = constp.tile([128, SHARD_PAD], F16)
            for t, s in [(waug, waug_in), (runr, runr_in), (bcol, bcol_in),
                         (linw, linw_in), (linb, linb_in), (iota, iota_in),
                         (idm, idm_in), (bl, bl_in), (ilo, ilo_in),
                         (ihi, ihi_in), (dstl, dstl_in), (xsT0, xsT0_in)]:
                nc.sync.dma_start(out=t[:], in_=s[:])

            negshift = constp.tile([128, 1], F32)
            nc.vector.memset(negshift[:], -EXP_SHIFT)
            eps = constp.tile([128, 1], F32, name="eps")
            nc.vector.memset(eps[:], 1e-6)
            adl = [constp.tile([128, WPC], F16, name=f"adl{i}") for i in range(2)]

            gt_bufs = [gathp.tile([128, CT_MAX, ROW], F16, name=f"gt{i}",
                                  tag=f"gt{i}") for i in range(NGT)]
            for gtb in gt_bufs:
                nc.vector.memset(gtb[:], 0.0)

            htab_own = [dram.tile([SHARD_PAD, ROW], F16, name=f"hown{i}",
                                  tag=f"hown{i}") for i in range(2)]
            htab_sh = [dram.tile([NCORES, SHARD_PAD, ROW], F16,
                                 addr_space="Shared", name=f"hsh{i}",
                                 tag=f"hsh{i}") for i in range(3)]
            pool_bi = dram.tile([128, 129], F32)
            pool_bo = dram.tile([128, 129], F32, addr_space="Shared")

            # ---- produce own-shard table rows for table `ti` from y [f, n] ----
            def own_rows(ti, w, y_ap):
                ps = psunx.tile([128, 129], F32, tag="unx")
                nc.tensor.matmul(ps[:], y_ap, waug[:, ti, :], start=True,
                                 stop=True, skip_group_check=True)
                st = stp.tile([128, 128], F16, tag="st")
                nc.scalar.activation(st[:], ps[:, 0:128], AF.Copy)
                nc.scalar.activation(adl[ti % 2][:, w:w + 1], ps[:, 128:129],
                                     AF.Copy)
                nc.sync.dma_start(
                    out=htab_own[ti % 2][w * 128:(w + 1) * 128, :]
                        .rearrange("(b p) e -> p b e", p=128),
                    in_=st[:].unsqueeze(1))

            htab_loc = [dram.tile([NCORES, SHARD_PAD, ROW], F16,
                                  name=f"hloc{i}", tag=f"hloc{i}")
                        for i in range(2)]

            def allgather(ti):
                nc.gpsimd.collective_compute(
                    "AllGather", OP.bypass, replica_groups=[list(range(NCORES))],
                    ins=[htab_own[ti % 2][:].opt()],
                    outs=[htab_sh[ti][:].opt()])
                nc.sync.dma_start(out=htab_loc[ti % 2][:], in_=htab_sh[ti][:])

            pool_ps = psp.tile([128, 129], F32, tag="pool", name="pool_ps")

            # ---- per-window output path for edge layer `layer` ----
            def window_out(layer, w, ps):
                dn = evp.tile([128, 1], F32, tag="dn")
                nc.scalar.activation(dn[:], ps[:, 128:129], AF.Relu, bias=eps[:])
                rc = evp.tile([128, 1], F32, tag="rc")
                nc.vector.reciprocal(rc[:], dn[:])
                xw = evp.tile([128, 128], F16, tag="xw")
                nc.scalar.activation(xw[:], ps[:, 0:128], AF.Copy, scale=rc[:])
                tp = pstr.tile([128, 128], F16, tag="tr")
                nc.tensor.transpose(tp[:], xw[:], idm[:])
                xwT = evp.tile([128, 128], F16, tag="xwT")
                nc.vector.tensor_copy(xwT[:], tp[:])
                up = psunx.tile([128, 129], F32, tag="unx", name=f"u_{layer}_{w}")
                nc.tensor.matmul(up[:, 0:128], runr[:, layer, :], xwT[:],
                                 start=True, stop=True, skip_group_check=True)
                y = evp.tile([128, 128], F16, tag="y")
                nc.scalar.activation(y[:], up[:, 0:128], AF.Relu,
                                     bias=bcol[:, layer:layer + 1])
                if layer < 2:
                    own_rows(layer + 1, w, y[:])
                else:
                    tp2 = pstr.tile([128, 128], F16, tag="tr",
                                    name=f"tr2_{w}")
                    nc.tensor.transpose(tp2[:], y[:], idm[:])
                    x1 = evp.tile([128, 129], F16, tag="x1")
                    nc.vector.tensor_copy(x1[:, 0:128], tp2[:])
                    nc.vector.memset(x1[:, 128:129], 1.0)
                    ob = evp.tile([128, 128], F16, tag="ob")
                    nc.vector.tensor_scalar(ob[:], iota[:], bl[:, w:w + 1],
                                            None, OP.is_equal)
                    nc.tensor.matmul(pool_ps[:], ob[:], x1[:],
                                     start=(w == 0), stop=(w == WPC - 1),
                                     skip_group_check=True)

            # ---- edge phase ----
            def edge_phase(layer):
                adl_cur = adl[layer % 2]
                off = {0: 0, 1: 0}
                slot_off = 0
                src_flat = htab_loc[layer % 2][:].rearrange("r s e -> (r s) e")
                for ch, meta in enumerate(chunks):
                    ct, tg0, tg1 = meta["ct"], meta["tg0"], meta["tg1"]
                    slots = meta["slots"]; ns = len(slots)
                    gt = gt_bufs[(layer * NCHUNK + ch) % NGT]
                    t0 = 0
                    for hi, tg in ((0, tg0), (1, tg1)):
                        n_seg = tg * 128
                        src_ap = (src_flat[LO_ROWS:NPAD, :] if hi
                                  else src_flat[0:LO_ROWS, :])
                        if hi:
                            idxs = ihi[:, off[1] // 16:(off[1] + n_seg) // 16]
                            off[1] += n_seg
                        else:
                            idxs = ilo[:, off[0] // 16:(off[0] + n_seg) // 16]
                            off[0] += n_seg
                        nc.gpsimd.dma_gather(
                            out_ap=gt[:, t0:t0 + tg, :], in_ap=src_ap,
                            idxs_ap=idxs, num_idxs=n_seg, num_idxs_reg=n_seg,
                            elem_size=ROW, single_packet=False,
                            queue_num=nc._gq[0] % NQUEUES)
                        nc._gq[0] += 1
                        t0 += tg
                    if edge_mode < 1:
                        slot_off += ns
                        continue
                    m0t = m0tp.tile([128, NS_MAX * 128], F8, tag="m0t")
                    nc.sync.dma_start(
                        out=m0t[:, 0:ns * 128],
                        in_=m0t_in[:, slot_off * 128:(slot_off + ns) * 128])
                    adx = psadx.tile([128, CT_MAX], F32, tag="adx")
                    tile_slots = {}
                    for si, (t, w) in enumerate(slots):
                        tile_slots.setdefault(t, []).append(si)
                    for t, sis in tile_slots.items():
                        for k, si in enumerate(sis):
                            _, w = slots[si]
                            nc.tensor.matmul(
                                adx[:, t:t + 1], m0t[:, si * 128:(si + 1) * 128],
                                adl_cur[:, w:w + 1], start=(k == 0),
                                stop=(k == len(sis) - 1), skip_group_check=True)
                    z  = ewp.tile([128, CT_MAX], F32, tag="z")
                    e1 = ewp.tile([128, CT_MAX], F32, tag="e1")
                    ef = ewp.tile([128, CT_MAX], F32, tag="ef")
                    nc.vector.tensor_tensor(z[:, 0:ct].unsqueeze(2),
                                            gt[:, 0:ct, 0:1],
                                            adx[:, 0:ct].unsqueeze(2), OP.add)
                    nc.scalar.activation(e1[:, 0:ct], z[:, 0:ct], AF.Exp,
                                         bias=negshift[:])
                    nc.scalar.activation(z[:, 0:ct], z[:, 0:ct], AF.Exp,
                                         bias=negshift[:], scale=0.2)
                    nc.vector.tensor_tensor(ef[:, 0:ct], e1[:, 0:ct],
                                            z[:, 0:ct], OP.max)
                    rhs = rhsp.tile([128, CT_MAX, 130], F16, tag="rhs")
                    nc.vector.tensor_tensor(
                        rhs[:, 0:ct, 0:128], gt[:, 0:ct, :],
                        ef[:, 0:ct].unsqueeze(2).to_broadcast((128, ct, 128)),
                        OP.mult)
                    nc.scalar.activation(rhs[:, 0:ct, 128:129],
                                         ef[:, 0:ct].unsqueeze(2), AF.Copy)
                    if edge_mode < 2:
                        slot_off += ns
                        continue
                    m0 = m0fp.tile([128, NS_MAX * 128], F8, tag="m0f")
                    nc.sync.dma_start(
                        out=m0[:, 0:ns * 128],
                        in_=m0f_in[:, slot_off * 128:(slot_off + ns) * 128])
                    wf, wl = {}, {}
                    for si, (t, w) in enumerate(slots):
                        if w not in wf:
                            wf[w] = si
                        wl[w] = si
                    psws = {w: psw.tile([128, 129], F32, tag="win",
                                        name=f"win{layer}_{ch}_{w}")
                            for w in wf}
                    for si, (t, w) in enumerate(slots):
                        nc.tensor.matmul(psws[w][:],
                                         m0[:, si * 128:(si + 1) * 128],
                                         rhs[:, t, 0:129],
                                         start=(si == wf[w]),
                                         stop=(si == wl[w]),
                                         skip_group_check=True)
                    if edge_mode >= 3:
                        for w in sorted(wf):
                            window_out(layer, w, psws[w])
                    slot_off += ns

            # ================= main =================
            for w in range(WPC):
                own_rows(0, w, xsT0[:, w * 128:(w + 1) * 128])
            allgather(0)
            for layer in range(n_layers):
                edge_phase(layer)
                if layer < 2 and edge_mode >= 3:
                    allgather(layer + 1)
            if dump_htab:
                hcp = stp.tile([128, dump_htab // 128, ROW], F16, tag="hcp")
                nc.sync.dma_start(
                    out=hcp[:],
                    in_=htab_loc[0][:].rearrange("r s e -> (r s) e")
                        [0:dump_htab, :].rearrange("(b p) e -> p b e", p=128))
                nc.sync.dma_start(
                    out=htab_out[:].rearrange("(b p) e -> p b e", p=128),
                    in_=hcp[:])
            if not with_pool or n_layers < 3 or edge_mode < 3:
                zz = evp.tile([128, C_OUT], F32, tag="res")
                nc.vector.memset(zz[:], 0.0)
                nc.sync.dma_start(out=out_t[:], in_=zz[:])
                return nc

            # ---- pool + final linear ----
            pooled = evp.tile([128, 129], F32, tag="pooled")
            nc.vector.tensor_copy(pooled[:], pool_ps[:])
            nc.sync.dma_start(out=pool_bi[:], in_=pooled[:])
            nc.gpsimd.collective_compute(
                "AllReduce", OP.add, replica_groups=[list(range(NCORES))],
                ins=[pool_bi[:].opt()], outs=[pool_bo[:].opt()])
            nc.sync.dma_start(out=pooled[:], in_=pool_bo[:])
            cnt = evp.tile([128, 1], F32, tag="cnt")
            nc.vector.tensor_scalar_max(cnt[:], pooled[:, 128:129], 1.0)
            rcn = evp.tile([128, 1], F32, tag="rcn")
            nc.vector.reciprocal(rcn[:], cnt[:])
            pm = evp.tile([128, 128], F16, tag="pm")
            nc.scalar.activation(pm[:], pooled[:, 0:128], AF.Copy, scale=rcn[:])
            pt = pstr.tile([128, 128], F16, tag="tr")
            nc.tensor.transpose(pt[:], pm[:], idm[:])
            pts = evp.tile([128, 128], F16, tag="pts")
            nc.vector.tensor_copy(pts[:], pt[:])
            ho = psw.tile([128, 129], F32, tag="win", name="ho")
            nc.tensor.matmul(ho[:, 0:C_OUT], pts[:], linw[:], start=True,
                             stop=True, skip_group_check=True)
            res = evp.tile([128, C_OUT], F32, tag="res")
            nc.vector.tensor_tensor(res[:], ho[:, 0:C_OUT], linb[:], OP.add)
            nc.sync.dma_start(out=out_t[:], in_=res[:])
    return nc


def run(inputs, trace=False, **build_kw):
    chunks, cores, NSLOT, n_lo, n_hi = prep_edges(np.asarray(inputs["edge_index"]))
    const_ins = make_weight_inputs(
        [np.asarray(inputs[f"W{i}"]) for i in (1, 2, 3)],
        [np.asarray(inputs[f"a_src{i}"]) for i in (1, 2, 3)],
        [np.asarray(inputs[f"a_dst{i}"]) for i in (1, 2, 3)],
        [np.asarray(inputs[f"b{i}"]) for i in (1, 2, 3)],
        np.asarray(inputs["lin_W"]), np.asarray(inputs["lin_b"]))
    batch = np.asarray(inputs["batch"])
    x = np.asarray(inputs["x"])

    nc = bacc.Bacc("TRN2", target_bir_lowering=False, debug=False,
                   num_devices=NCORES, num_swdge_queues=NQUEUES)
    nc._gq = [0]
    build(nc, chunks, NSLOT, n_lo, n_hi, **build_kw)
    nc.compile()
    split_waits(nc)

    in_maps = []
    for c in range(NCORES):
        m = dict(const_ins)
        m["batchl"] = make_batch_input(batch, c)
        m["xsT0"] = make_xsT0(x, c)
        m["idxlo"] = cores[c]["idxlo"]
        m["idxhi"] = cores[c]["idxhi"]
        m["dstl"] = cores[c]["dstl"]
        m["m0t"] = cores[c]["m0t"]
        m["m0f"] = cores[c]["m0f"]
        in_maps.append(m)
    res = bass_utils.run_bass_kernel_spmd(nc, in_maps,
                                          core_ids=list(range(NCORES)),
                                          trace=trace)
    return res.results[0], res


def kernel(**inputs):
    """Harness entry: full unsharded inputs -> [128, 10] fp32 output."""
    out, _ = run(inputs)
    if isinstance(out, dict):
        out = out["out"]
    return np.asarray(out, dtype=np.float32)
